# revision 1
# baseline (speedup 1.0000x reference)
"""LIF spike kernel (T-step leaky integrate-and-fire recurrence) on 8 TRN2 cores.

Reference semantics (per element, thre = tanh(w[c])):
    u_t = TAU * u_{t-1} * (1 - o_{t-1}) + x_t
    o_t = (u_t - thre > 0) ? 1.0 : 0.0

Raw-bass implementation (no Tile — this walrus build allows only one sync
wait per compute instruction, so waits are standalone wait_ge instructions).

Per step, carrying M_t = u_t * (u_t <= thre):
    DVE:  U  = (M * TAU) + X_t          scalar_tensor_tensor (mult, add)
    DVE:  M  = (U <= thre) * U          scalar_tensor_tensor (is_le, mult)
    ACT:  SG = Sign(U - thre)           activation Sign, bias = -tanh(w)
    ACT:  O  = Relu(SG) -> uint8        exact 0/1 spikes
    ACT:  dma o[t] <- O
All products are by 1.0/0.0 masks or by TAU=0.25 (a power of two), and the
compare path matches the reference's (u - thre > 0), so the result is
bit-exact vs the fp32 reference.

Sharding: B=32 split across 8 cores (4 each).  Per-core SBUF layout:
partition p = bp*64 + c (bp = batch pair, c = channel), free f = bf*1024 + hw,
with b = bp*2 + bf.  The host pre-transposes x so each timestep is one
contiguous [128, 2048] fp32 DMA; spikes return as uint8 and are cast on host.
"""

import numpy as np

import concourse.bass as bass
import concourse.mybir as mybir
from concourse.bass_utils import run_bass_kernel_spmd

TAU = 0.25
T, B, C, H, W = 16, 32, 64, 32, 32
N_CORES = 8
B_PER = B // N_CORES  # 4
HWF = H * W  # 1024
P = 128  # partitions: 2 batch-pairs x 64 channels
FD = (B_PER // 2) * HWF  # 2048 free-dim elements per partition per step

XS = 3  # X double-buffer slots
US = 2  # U slots
OS = 8  # O slots

_cache = {}
last_results = None  # BassKernelResults of the most recent run (for test harness)


def _build_nc():
    nc = bass.Bass("TRN2", target_bir_lowering=False, debug=False, num_devices=N_CORES)
    f32 = mybir.dt.float32
    u8 = mybir.dt.uint8
    x_d = nc.dram_tensor("x", [T, P, FD], f32, kind="ExternalInput").ap()
    w_d = nc.dram_tensor("w", [P, 1], f32, kind="ExternalInput").ap()
    o_d = nc.dram_tensor("o", [T, P, FD], u8, kind="ExternalOutput").ap()

    AT = mybir.AluOpType
    AF = mybir.ActivationFunctionType

    X = nc.alloc_sbuf_tensor("Xb", [P, XS * FD], f32).ap()
    U = nc.alloc_sbuf_tensor("Ub", [P, US * FD], f32).ap()
    M = nc.alloc_sbuf_tensor("Mb", [P, FD], f32).ap()
    SG = nc.alloc_sbuf_tensor("SGb", [P, FD], f32).ap()
    O = nc.alloc_sbuf_tensor("Ob", [P, OS * FD], u8).ap()
    WT = nc.alloc_sbuf_tensor("WTb", [P, 1], f32).ap()
    NT = nc.alloc_sbuf_tensor("NTb", [P, 1], f32).ap()  # -tanh(w)
    TH = nc.alloc_sbuf_tensor("THb", [P, 1], f32).ap()  # +tanh(w)

    def xsl(t):
        return X[:, (t % XS) * FD : (t % XS + 1) * FD]

    def usl(t):
        return U[:, (t % US) * FD : (t % US + 1) * FD]

    def osl(t):
        return O[:, (t % OS) * FD : (t % OS + 1) * FD]

    import contextlib

    with contextlib.ExitStack() as st:
        block = st.enter_context(nc.Block())
        dve = st.enter_context(nc.semaphore("dve"))
        act = st.enter_context(nc.semaphore("act"))
        dw = st.enter_context(nc.semaphore("dw"))
        # one sem per SBUF slot -> never more than one outstanding inc per sem,
        # so count-based waits are unambiguous under out-of-order DMA completion
        dx = [st.enter_context(nc.semaphore(f"dx{i}")) for i in range(XS)]
        do = [st.enter_context(nc.semaphore(f"do{i}")) for i in range(OS)]

        @block.sync
        def _(sp):
            sp.dma_start(out=WT, in_=w_d).then_inc(dw, 16)
            for t in range(T):
                if t >= XS:
                    sp.wait_ge(dve, t - XS + 1)  # STT2(t-XS) read its X slot
                sp.dma_start(out=xsl(t), in_=x_d[t]).then_inc(dx[t % XS], 16)

        @block.scalar
        def _(ac):
            ac.wait_ge(dw, 16)
            ac.activation(NT, WT, AF.Tanh, scale=-1.0)  # tanh odd: -tanh(w)
            ac.activation(TH, WT, AF.Tanh).then_inc(act, 1)
            ac.drain()
            for t in range(T):
                ac.wait_ge(dve, t + 1)  # U(t) ready
                ac.activation(SG, usl(t), AF.Sign, bias=NT).then_inc(act, 1)
                if t >= OS:
                    ac.wait_ge(do[t % OS], 16 * (t // OS))  # O slot drained
                ac.drain()
                ac.activation(osl(t), SG, AF.Relu)
                ac.drain()
                ac.dma_start(out=o_d[t], in_=osl(t)).then_inc(do[t % OS], 16)
            for i in range(OS):
                n_dmas = len([t for t in range(T) if t % OS == i])
                ac.wait_ge(do[i], 16 * n_dmas)

        @block.vector
        def _(dv):
            dv.wait_ge(act, 1)  # thre ready
            dv.memset(M, 0.0)
            dv.drain()
            for t in range(T):
                dv.wait_ge(dx[t % XS], 16 * (t // XS + 1))  # X(t) loaded
                if t >= US:
                    dv.wait_ge(act, t)  # Sign(t-US) read its U slot
                dv.scalar_tensor_tensor(
                    usl(t), M, TAU, xsl(t), AT.mult, AT.add
                ).then_inc(dve, 1)
                dv.drain()
                dv.scalar_tensor_tensor(M, usl(t), TH, usl(t), AT.is_le, AT.mult)
                dv.drain()

    return nc


def _get_nc():
    if "nc" not in _cache:
        _cache["nc"] = _build_nc()
    return _cache["nc"]


def _shard_x(x):
    """x [T,B,C,H,W] fp32 -> list of 8 contiguous [T,128,2048] arrays."""
    xf = x.reshape(T, B, C, HWF)
    shards = []
    for i in range(N_CORES):
        xc = xf[:, i * B_PER : (i + 1) * B_PER]  # [T,4,C,1024]
        xc = xc.reshape(T, 2, 2, C, HWF).transpose(0, 1, 3, 2, 4)  # t,bp,c,bf,f
        shards.append(np.ascontiguousarray(xc).reshape(T, P, FD))
    return shards


def _unshard_o(per_core):
    """list of 8 [T,128,2048] uint8 -> [T,B,C,H,W] fp32."""
    outs = []
    for oc in per_core:
        oc = oc.reshape(T, 2, C, 2, HWF).transpose(0, 1, 3, 2, 4)  # t,bp,bf,c,f
        outs.append(oc.reshape(T, B_PER, C, H, W))
    return np.concatenate(outs, axis=1).astype(np.float32)


def kernel(x, w):
    global last_results
    x = np.ascontiguousarray(np.asarray(x), dtype=np.float32)
    w = np.tile(np.asarray(w, dtype=np.float32).reshape(64, 1), (2, 1))  # [128,1]

    nc = _get_nc()
    shards = _shard_x(x)
    in_maps = [{"x": shards[i], "w": w} for i in range(N_CORES)]
    last_results = run_bass_kernel_spmd(nc, in_maps, core_ids=list(range(N_CORES)))
    return _unshard_o([last_results.results[i]["o"] for i in range(N_CORES)])



# revision 4
# speedup vs baseline: 1.5683x; 1.5683x over previous
"""LIF spike kernel (T-step leaky integrate-and-fire recurrence) on 8 TRN2 cores.

Reference semantics (per element, thre = tanh(w[c])):
    u_t = TAU * u_{t-1} * (1 - o_{t-1}) + x_t
    o_t = (u_t - thre > 0) ? 1.0 : 0.0

This version trades bit-exactness for speed (harness gate is rel_err < 2e-2):
x and the carried state are fp16, which measures rel_err ~1.1e-2 on the
reference inputs and unlocks the DVE 2x/4x perf modes plus half the input DMA
traffic vs fp32.

Per step, full 2048-col width, all on DVE (3 ops instead of the 2 1x-mode
scalar_tensor_tensor ops of the fp32 version -- TT runs at 2x and TS at 4x for
fp16, so 3 fast ops beat 2 slow ones), with carry C = TAU*u*(u<=thre):
    TT : U = C + X_t                 (tensor_tensor add, 2x)
    TS : z = (U <= thre) * TAU       (tensor_scalar, 4x; z in {0, TAU})
    TT : C = z * U                   (2x; TAU = 2^-2 so the scale is exact)
ACT computes spikes in one op: O = Sign(U - thre) -> int8 {-1,0,1}; the host
maps spike = (O == 1).  ACT also issues the O store DMAs so the SP queue only
issues X loads and never stalls the load prefetch behind the Sign chain.

Sharding: B=32 split across 8 cores (4 each).  Per-core SBUF layout:
partition p = bp*64 + c (bp = batch pair, c = channel), free f = bf*1024 + hw,
with b = bp*2 + bf.  The host pre-transposes x to fp16 so each timestep is one
contiguous [128, 2048] DMA; spikes return as int8 and are mapped on host.
"""

import numpy as np

import concourse.bass as bass
import concourse.mybir as mybir
from concourse.bass_utils import run_bass_kernel_spmd

TAU = 0.25
T, B, C, H, W = 16, 32, 64, 32, 32
N_CORES = 8
B_PER = B // N_CORES  # 4
HWF = H * W  # 1024
P = 128  # partitions: 2 batch-pairs x 64 channels
FD = (B_PER // 2) * HWF  # 2048 free-dim elements per partition per step

XS = 4  # X buffer slots
US = 2  # U slots
OS = 4  # O slots

_cache = {}
last_results = None  # BassKernelResults of the most recent run (for test harness)


def _build_nc():
    nc = bass.Bass("TRN2", target_bir_lowering=False, debug=False, num_devices=N_CORES)
    f32 = mybir.dt.float32
    f16 = mybir.dt.float16
    i8 = mybir.dt.int8
    x_d = nc.dram_tensor("x", [T, P, FD], f16, kind="ExternalInput").ap()
    w_d = nc.dram_tensor("w", [P, 1], f32, kind="ExternalInput").ap()
    o_d = nc.dram_tensor("o", [T, P, FD], i8, kind="ExternalOutput").ap()

    AT = mybir.AluOpType
    AF = mybir.ActivationFunctionType

    X = nc.alloc_sbuf_tensor("Xb", [P, XS * FD], f16).ap()
    U = nc.alloc_sbuf_tensor("Ub", [P, US * FD], f16).ap()
    Cc = nc.alloc_sbuf_tensor("Cb", [P, FD], f16).ap()
    Z = nc.alloc_sbuf_tensor("Zb", [P, FD], f16).ap()
    O = nc.alloc_sbuf_tensor("Ob", [P, OS * FD], i8).ap()
    WT = nc.alloc_sbuf_tensor("WTb", [P, 1], f32).ap()
    NT = nc.alloc_sbuf_tensor("NTb", [P, 1], f32).ap()  # -tanh(w)
    TH = nc.alloc_sbuf_tensor("THb", [P, 1], f32).ap()  # +tanh(w)

    def xsl(t):
        return X[:, (t % XS) * FD : (t % XS + 1) * FD]

    def usl(t):
        return U[:, (t % US) * FD : (t % US + 1) * FD]

    def osl(t):
        return O[:, (t % OS) * FD : (t % OS + 1) * FD]

    import contextlib

    with contextlib.ExitStack() as st:
        block = st.enter_context(nc.Block())
        udve = st.enter_context(nc.semaphore("udve"))  # t+1 after TT-U(t)
        act = st.enter_context(nc.semaphore("act"))  # 1 + (t+1) after Sign(t)
        dw = st.enter_context(nc.semaphore("dw"))
        dx = [st.enter_context(nc.semaphore(f"dx{i}")) for i in range(XS)]
        do = [st.enter_context(nc.semaphore(f"do{i}")) for i in range(OS)]

        @block.sync
        def _(sp):
            sp.dma_start(out=WT, in_=w_d).then_inc(dw, 16)
            for t in range(T):
                if t >= XS:
                    # X slot's previous tenant (t-XS) was consumed by TT-U
                    sp.wait_ge(udve, t - XS + 1)
                sp.dma_start(out=xsl(t), in_=x_d[t]).then_inc(dx[t % XS], 16)

        @block.scalar
        def _(ac):
            ac.wait_ge(dw, 16)
            ac.activation(NT, WT, AF.Tanh, scale=-1.0)  # tanh odd: -tanh(w)
            ac.activation(TH, WT, AF.Tanh).then_inc(act, 1)
            ac.drain()
            for t in range(T):
                ac.wait_ge(udve, t + 1)  # U(t) ready
                if t >= OS:
                    ac.wait_ge(do[t % OS], 16 * (t // OS))  # O slot drained
                ac.activation(osl(t), usl(t), AF.Sign, bias=NT).then_inc(act, 1)
                ac.drain()  # DGE reads SBUF async: Sign must land before the store
                ac.dma_start(out=o_d[t], in_=osl(t)).then_inc(do[t % OS], 16)
            for i in range(OS):
                n_dmas = len([t for t in range(T) if t % OS == i])
                ac.wait_ge(do[i], 16 * n_dmas)

        @block.vector
        def _(dv):
            dv.wait_ge(act, 1)  # thre ready
            dv.memset(Cc, 0.0)
            for t in range(T):
                dv.wait_ge(dx[t % XS], 16 * (t // XS + 1))  # X(t) loaded
                if t >= US:
                    dv.wait_ge(act, t)  # Sign(t-2) read its U slot
                dv.tensor_tensor(usl(t), Cc, xsl(t), AT.add).then_inc(udve, 1)
                dv.tensor_scalar(Z, usl(t), TH, TAU, AT.is_le, AT.mult)
                dv.tensor_tensor(Cc, Z, usl(t), AT.mult)

    return nc


def _get_nc():
    if "nc" not in _cache:
        _cache["nc"] = _build_nc()
    return _cache["nc"]


def _shard_x(x):
    """x [T,B,C,H,W] fp32 -> list of 8 contiguous [T,128,2048] fp16 arrays."""
    xf = x.reshape(T, B, C, HWF)
    shards = []
    for i in range(N_CORES):
        xc = xf[:, i * B_PER : (i + 1) * B_PER]  # [T,4,C,1024]
        xc = xc.reshape(T, 2, 2, C, HWF).transpose(0, 1, 3, 2, 4)  # t,bp,c,bf,f
        shards.append(np.ascontiguousarray(xc).reshape(T, P, FD).astype(np.float16))
    return shards


def _unshard_o(per_core):
    """list of 8 [T,128,2048] int8 sign values -> [T,B,C,H,W] fp32 spikes."""
    outs = []
    for oc in per_core:
        oc = (oc == 1).astype(np.float32)
        oc = oc.reshape(T, 2, C, 2, HWF).transpose(0, 1, 3, 2, 4)  # t,bp,bf,c,f
        outs.append(oc.reshape(T, B_PER, C, H, W))
    return np.concatenate(outs, axis=1)


def kernel(x, w):
    global last_results
    x = np.ascontiguousarray(np.asarray(x), dtype=np.float32)
    w = np.tile(np.asarray(w, dtype=np.float32).reshape(64, 1), (2, 1))  # [128,1]

    nc = _get_nc()
    shards = _shard_x(x)
    in_maps = [{"x": shards[i], "w": w} for i in range(N_CORES)]
    last_results = run_bass_kernel_spmd(nc, in_maps, core_ids=list(range(N_CORES)))
    return _unshard_o([last_results.results[i]["o"] for i in range(N_CORES)])


# revision 5
# speedup vs baseline: 1.6657x; 1.0621x over previous
"""LIF spike kernel (T-step leaky integrate-and-fire recurrence) on 8 TRN2 cores.

Reference semantics (per element, thre = tanh(w[c])):
    u_t = TAU * u_{t-1} * (1 - o_{t-1}) + x_t
    o_t = (u_t - thre > 0) ? 1.0 : 0.0

This version trades bit-exactness for speed (harness gate is rel_err < 2e-2):
x and the carried state are fp16, which measures rel_err ~1.1e-2 on the
reference inputs and unlocks the DVE 2x/4x perf modes plus half the input DMA
traffic vs fp32.

Per step, with carry C = TAU*u*(u<=thre), the fp32 STT pair of the baseline is
replaced by a 3-op fp16 form (TT runs at 2x and TS at 4x; STT is stuck at 1x):
    TT : U = C + X_t                 (tensor_tensor add)
    TS : z = (U <= thre) * TAU       (tensor_scalar; z in {0, TAU})
    TT : C = z * U                   (TAU = 2^-2 so the scale is exact)
The free dim (2048 cols) is split: DVE runs all 3 ops on cols [0,DCOL); for
cols [DCOL,2048) the gpsimd/Pool engine runs the TT add (the only ALU op
walrus accepts on Pool) in 2 chunks while DVE does that slice's TS/TT-mult —
the chunking pipelines the pool->DVE ping-pong so it stays off the critical
path.  ACT computes spikes in one op: O = Sign(U - thre) -> int8 {-1,0,1};
host maps spike = (O == 1).  O stores are issued from the SP queue lagged 3
steps behind the X loads so the store's act-semaphore wait never stalls the
load prefetch (and no engine drain is needed: the store is semaphore-ordered
after Sign).

Sharding: B=32 split across 8 cores (4 each).  Per-core SBUF layout:
partition p = bp*64 + c (bp = batch pair, c = channel), free f = bf*1024 + hw,
with b = bp*2 + bf.  The host pre-transposes x to fp16 so each timestep is one
contiguous [128, 2048] DMA; spikes return as int8 and are mapped on host.
"""

import numpy as np

import concourse.bass as bass
import concourse.mybir as mybir
from concourse.bass_utils import run_bass_kernel_spmd

TAU = 0.25
T, B, C, H, W = 16, 32, 64, 32, 32
N_CORES = 8
B_PER = B // N_CORES  # 4
HWF = H * W  # 1024
P = 128  # partitions: 2 batch-pairs x 64 channels
FD = (B_PER // 2) * HWF  # 2048 free-dim elements per partition per step

DCOL = 1100  # DVE-only slice; cols [DCOL, FD) get their TT-add from Pool
NCHUNK = 2  # pool slice chunks (pipelines the pool->DVE handoff)
XS = 4  # X buffer slots
US = 2  # U slots
OS = 6  # O slots
LAG = 3  # store O(t-LAG) after loading X(t) on the SP queue

_cache = {}
last_results = None  # BassKernelResults of the most recent run (for test harness)


def _build_nc():
    nc = bass.Bass("TRN2", target_bir_lowering=False, debug=False, num_devices=N_CORES)
    f32 = mybir.dt.float32
    f16 = mybir.dt.float16
    i8 = mybir.dt.int8
    x_d = nc.dram_tensor("x", [T, P, FD], f16, kind="ExternalInput").ap()
    w_d = nc.dram_tensor("w", [P, 1], f32, kind="ExternalInput").ap()
    o_d = nc.dram_tensor("o", [T, P, FD], i8, kind="ExternalOutput").ap()

    AT = mybir.AluOpType
    AF = mybir.ActivationFunctionType

    d = DCOL
    s = FD - d
    cuts = [d + (s * i) // NCHUNK for i in range(NCHUNK + 1)]

    X = nc.alloc_sbuf_tensor("Xb", [P, XS * FD], f16).ap()
    U = nc.alloc_sbuf_tensor("Ub", [P, US * FD], f16).ap()
    Cc = nc.alloc_sbuf_tensor("Cb", [P, FD], f16).ap()
    Z = nc.alloc_sbuf_tensor("Zb", [P, FD], f16).ap()
    O = nc.alloc_sbuf_tensor("Ob", [P, OS * FD], i8).ap()
    WT = nc.alloc_sbuf_tensor("WTb", [P, 1], f32).ap()
    NT = nc.alloc_sbuf_tensor("NTb", [P, 1], f32).ap()  # -tanh(w)
    TH = nc.alloc_sbuf_tensor("THb", [P, 1], f32).ap()  # +tanh(w)

    def xsl(t, lo, hi):
        return X[:, (t % XS) * FD + lo : (t % XS) * FD + hi]

    def usl(t, lo, hi):
        return U[:, (t % US) * FD + lo : (t % US) * FD + hi]

    def osl(t):
        return O[:, (t % OS) * FD : (t % OS + 1) * FD]

    import contextlib

    with contextlib.ExitStack() as st:
        block = st.enter_context(nc.Block())
        udve = st.enter_context(nc.semaphore("udve"))  # t+1 after TT-U_d(t)
        act = st.enter_context(nc.semaphore("act"))  # 1 + (t+1) after Sign(t)
        dw = st.enter_context(nc.semaphore("dw"))
        up = [st.enter_context(nc.semaphore(f"up{i}")) for i in range(NCHUNK)]
        cd = [st.enter_context(nc.semaphore(f"cd{i}")) for i in range(NCHUNK)]
        dx = [st.enter_context(nc.semaphore(f"dx{i}")) for i in range(XS)]
        do = [st.enter_context(nc.semaphore(f"do{i}")) for i in range(OS)]

        @block.sync
        def _(sp):
            sp.dma_start(out=WT, in_=w_d).then_inc(dw, 16)
            for t in range(T):
                if t >= XS:
                    # X slot's previous tenant (t-XS) consumed by both engines
                    sp.wait_ge(udve, t - XS + 1)
                    sp.wait_ge(up[NCHUNK - 1], t - XS + 1)
                sp.dma_start(out=xsl(t, 0, FD), in_=x_d[t]).then_inc(dx[t % XS], 16)
                if t >= LAG:
                    sp.wait_ge(act, t - LAG + 2)  # Sign(t-LAG) done
                    sp.dma_start(out=o_d[t - LAG], in_=osl(t - LAG)).then_inc(
                        do[(t - LAG) % OS], 16
                    )
            for t in range(T - LAG, T):
                sp.wait_ge(act, t + 2)
                sp.dma_start(out=o_d[t], in_=osl(t)).then_inc(do[t % OS], 16)
            for i in range(OS):
                n_dmas = len([t for t in range(T) if t % OS == i])
                sp.wait_ge(do[i], 16 * n_dmas)

        @block.scalar
        def _(ac):
            ac.wait_ge(dw, 16)
            ac.activation(NT, WT, AF.Tanh, scale=-1.0)  # tanh odd: -tanh(w)
            ac.activation(TH, WT, AF.Tanh).then_inc(act, 1)
            ac.drain()
            for t in range(T):
                ac.wait_ge(udve, t + 1)  # U[:, :d](t) ready
                ac.wait_ge(up[NCHUNK - 1], t + 1)  # U[:, d:](t) ready
                if t >= OS:
                    ac.wait_ge(do[t % OS], 16 * (t // OS))  # O slot drained
                ac.activation(osl(t), usl(t, 0, FD), AF.Sign, bias=NT).then_inc(act, 1)

        @block.vector
        def _(dv):
            dv.memset(Cc[:, 0:d], 0.0)
            dv.wait_ge(act, 1)  # thre ready
            for t in range(T):
                dv.wait_ge(dx[t % XS], 16 * (t // XS + 1))  # X(t) loaded
                if t >= US:
                    dv.wait_ge(act, t - US + 2)  # Sign(t-US) read its U slot
                dv.tensor_tensor(usl(t, 0, d), Cc[:, 0:d], xsl(t, 0, d), AT.add).then_inc(
                    udve, 1
                )
                for i in range(NCHUNK):
                    lo, hi = cuts[i], cuts[i + 1]
                    dv.wait_ge(up[i], t + 1)  # pool chunk's U ready
                    dv.tensor_scalar(
                        Z[:, lo:hi], usl(t, lo, hi), TH, TAU, AT.is_le, AT.mult
                    )
                    dv.tensor_tensor(
                        Cc[:, lo:hi], Z[:, lo:hi], usl(t, lo, hi), AT.mult
                    ).then_inc(cd[i], 1)
                dv.tensor_scalar(Z[:, 0:d], usl(t, 0, d), TH, TAU, AT.is_le, AT.mult)
                dv.tensor_tensor(Cc[:, 0:d], Z[:, 0:d], usl(t, 0, d), AT.mult)

        @block.gpsimd
        def _(pl):
            pl.memset(Cc[:, d:FD], 0.0)
            for t in range(T):
                pl.wait_ge(dx[t % XS], 16 * (t // XS + 1))  # X(t) loaded
                for i in range(NCHUNK):
                    lo, hi = cuts[i], cuts[i + 1]
                    if t >= 1:
                        pl.wait_ge(cd[i], t)  # C chunk from t-1 ready
                    if t >= US:
                        pl.wait_ge(act, t - US + 2)  # Sign(t-US) read U slot
                    pl.tensor_tensor(
                        usl(t, lo, hi), Cc[:, lo:hi], xsl(t, lo, hi), AT.add
                    ).then_inc(up[i], 1)

    return nc


def _get_nc():
    if "nc" not in _cache:
        _cache["nc"] = _build_nc()
    return _cache["nc"]


def _shard_x(x):
    """x [T,B,C,H,W] fp32 -> list of 8 contiguous [T,128,2048] fp16 arrays."""
    xf = x.reshape(T, B, C, HWF)
    shards = []
    for i in range(N_CORES):
        xc = xf[:, i * B_PER : (i + 1) * B_PER]  # [T,4,C,1024]
        xc = xc.reshape(T, 2, 2, C, HWF).transpose(0, 1, 3, 2, 4)  # t,bp,c,bf,f
        shards.append(np.ascontiguousarray(xc).reshape(T, P, FD).astype(np.float16))
    return shards


def _unshard_o(per_core):
    """list of 8 [T,128,2048] int8 sign values -> [T,B,C,H,W] fp32 spikes."""
    outs = []
    for oc in per_core:
        oc = (oc == 1).astype(np.float32)
        oc = oc.reshape(T, 2, C, 2, HWF).transpose(0, 1, 3, 2, 4)  # t,bp,bf,c,f
        outs.append(oc.reshape(T, B_PER, C, H, W))
    return np.concatenate(outs, axis=1)


def kernel(x, w):
    global last_results
    x = np.ascontiguousarray(np.asarray(x), dtype=np.float32)
    w = np.tile(np.asarray(w, dtype=np.float32).reshape(64, 1), (2, 1))  # [128,1]

    nc = _get_nc()
    shards = _shard_x(x)
    in_maps = [{"x": shards[i], "w": w} for i in range(N_CORES)]
    last_results = run_bass_kernel_spmd(nc, in_maps, core_ids=list(range(N_CORES)))
    return _unshard_o([last_results.results[i]["o"] for i in range(N_CORES)])


# revision 6
# speedup vs baseline: 1.7408x; 1.0451x over previous
"""LIF spike kernel (T-step leaky integrate-and-fire recurrence) on 8 TRN2 cores.

Reference semantics (per element, thre = tanh(w[c])):
    u_t = TAU * u_{t-1} * (1 - o_{t-1}) + x_t
    o_t = (u_t - thre > 0) ? 1.0 : 0.0

This version trades bit-exactness for speed (harness gate is rel_err < 2e-2):
x and the carried state are fp16, which measures rel_err ~1.1e-2 on the
reference inputs and unlocks the DVE 2x/4x perf modes plus half the input DMA
traffic vs fp32.

Per step, with carry C = TAU*u*(u<=thre), the fp32 STT pair of the baseline is
replaced by a 3-op fp16 form (fp16 tensor_tensor runs at 2x and tensor_scalar
at 4x, while scalar_tensor_tensor is stuck at 1x):
    TT : U = C + X_t                 (tensor_tensor add)
    TS : z = (U <= thre) * TAU       (tensor_scalar; z in {0, TAU})
    TT : C = z * U                   (TAU = 2^-2 so the scale is exact)
Work split: DVE runs all 3 ops on cols [0,DCOL) plus the TS compare for the
rest; the gpsimd/Pool engine runs the TT add AND TT mult for cols [DCOL,2048)
(walrus only accepts add/mult TTs on Pool) in 2 chunks, ordered adds-first
then mults so the pool->DVE z-handoff pipelines instead of serializing.
ACT computes spikes as two Sign ops (one per slice, so each fires as soon as
its slice's U is ready): O = Sign(U - thre) -> int8 {-1,0,1}; the host maps
spike = (O == 1).  O stores are issued from the SP queue lagged LAG steps
behind the X loads so their semaphore waits never stall the load prefetch.

Sharding: B=32 split across 8 cores (4 each).  Per-core SBUF layout:
partition p = bp*64 + c (bp = batch pair, c = channel), free f = bf*1024 + hw,
with b = bp*2 + bf.  The host pre-transposes x to fp16 so each timestep is one
contiguous [128, 2048] DMA; spikes return as int8 and are mapped on host.
"""

import numpy as np

import concourse.bass as bass
import concourse.mybir as mybir
from concourse.bass_utils import run_bass_kernel_spmd

TAU = 0.25
T, B, C, H, W = 16, 32, 64, 32, 32
N_CORES = 8
B_PER = B // N_CORES  # 4
HWF = H * W  # 1024
P = 128  # partitions: 2 batch-pairs x 64 channels
FD = (B_PER // 2) * HWF  # 2048 free-dim elements per partition per step

DCOL = 1600  # DVE-only slice; cols [DCOL, FD) are computed by Pool (+DVE TS)
XS = 4  # X buffer slots
US = 2  # U slots
OS = 6  # O slots
LAG = 3  # store O(t-LAG) after loading X(t) on the SP queue

_cache = {}
last_results = None  # BassKernelResults of the most recent run (for test harness)


def _build_nc():
    nc = bass.Bass("TRN2", target_bir_lowering=False, debug=False, num_devices=N_CORES)
    f32 = mybir.dt.float32
    f16 = mybir.dt.float16
    i8 = mybir.dt.int8
    x_d = nc.dram_tensor("x", [T, P, FD], f16, kind="ExternalInput").ap()
    w_d = nc.dram_tensor("w", [P, 1], f32, kind="ExternalInput").ap()
    o_d = nc.dram_tensor("o", [T, P, FD], i8, kind="ExternalOutput").ap()

    AT = mybir.AluOpType
    AF = mybir.ActivationFunctionType

    d = DCOL
    s = FD - d
    cuts = [d, d + s // 2, FD]

    X = nc.alloc_sbuf_tensor("Xb", [P, XS * FD], f16).ap()
    U = nc.alloc_sbuf_tensor("Ub", [P, US * FD], f16).ap()
    Cc = nc.alloc_sbuf_tensor("Cb", [P, FD], f16).ap()
    Z = nc.alloc_sbuf_tensor("Zb", [P, FD], f16).ap()
    O = nc.alloc_sbuf_tensor("Ob", [P, OS * FD], i8).ap()
    WT = nc.alloc_sbuf_tensor("WTb", [P, 1], f32).ap()
    NT = nc.alloc_sbuf_tensor("NTb", [P, 1], f32).ap()  # -tanh(w)
    TH = nc.alloc_sbuf_tensor("THb", [P, 1], f32).ap()  # +tanh(w)

    def xsl(t, lo, hi):
        return X[:, (t % XS) * FD + lo : (t % XS) * FD + hi]

    def usl(t, lo, hi):
        return U[:, (t % US) * FD + lo : (t % US) * FD + hi]

    def osl(t, lo=0, hi=FD):
        return O[:, (t % OS) * FD + lo : (t % OS) * FD + hi]

    import contextlib

    with contextlib.ExitStack() as st:
        block = st.enter_context(nc.Block())
        udve = st.enter_context(nc.semaphore("udve"))  # t+1 after TT-U_d(t)
        act = st.enter_context(nc.semaphore("act"))  # 1 + (t+1) after Sign_d(t)
        acs = st.enter_context(nc.semaphore("acs"))  # t+1 after Sign_s(t)
        dw = st.enter_context(nc.semaphore("dw"))
        up = [st.enter_context(nc.semaphore(f"up{i}")) for i in range(2)]
        zd = [st.enter_context(nc.semaphore(f"zd{i}")) for i in range(2)]
        cm = [st.enter_context(nc.semaphore(f"cm{i}")) for i in range(2)]
        dx = [st.enter_context(nc.semaphore(f"dx{i}")) for i in range(XS)]
        do = [st.enter_context(nc.semaphore(f"do{i}")) for i in range(OS)]

        @block.sync
        def _(sp):
            for t in range(T):
                if t >= XS:
                    # X slot's previous tenant (t-XS) consumed by both engines
                    sp.wait_ge(udve, t - XS + 1)
                    sp.wait_ge(up[1], t - XS + 1)
                sp.dma_start(out=xsl(t, 0, FD), in_=x_d[t]).then_inc(dx[t % XS], 16)
                if t == 0:
                    sp.dma_start(out=WT, in_=w_d).then_inc(dw, 16)
                if t >= LAG:
                    tt = t - LAG
                    sp.wait_ge(act, tt + 2)  # Sign_d(tt) done
                    sp.wait_ge(acs, tt + 1)  # Sign_s(tt) done
                    sp.dma_start(out=o_d[tt], in_=osl(tt)).then_inc(do[tt % OS], 16)
            for t in range(T - LAG, T):
                sp.wait_ge(act, t + 2)
                sp.wait_ge(acs, t + 1)
                sp.dma_start(out=o_d[t], in_=osl(t)).then_inc(do[t % OS], 16)
            for i in range(OS):
                n_dmas = len([t for t in range(T) if t % OS == i])
                sp.wait_ge(do[i], 16 * n_dmas)

        @block.scalar
        def _(ac):
            ac.wait_ge(dw, 16)
            ac.activation(NT, WT, AF.Tanh, scale=-1.0)  # tanh odd: -tanh(w)
            ac.activation(TH, WT, AF.Tanh).then_inc(act, 1)
            ac.drain()
            for t in range(T):
                ac.wait_ge(udve, t + 1)  # U[:, :d](t) ready
                if t >= OS:
                    ac.wait_ge(do[t % OS], 16 * (t // OS))  # O slot drained
                ac.activation(osl(t, 0, d), usl(t, 0, d), AF.Sign, bias=NT).then_inc(
                    act, 1
                )
                ac.wait_ge(up[1], t + 1)  # U[:, d:](t) ready
                ac.activation(osl(t, d, FD), usl(t, d, FD), AF.Sign, bias=NT).then_inc(
                    acs, 1
                )

        @block.vector
        def _(dv):
            dv.memset(Cc[:, 0:d], 0.0)
            dv.wait_ge(act, 1)  # thre ready
            for t in range(T):
                dv.wait_ge(dx[t % XS], 16 * (t // XS + 1))  # X(t) loaded
                if t >= US:
                    dv.wait_ge(act, t)  # Sign_d(t-2) read its U slot
                dv.tensor_tensor(usl(t, 0, d), Cc[:, 0:d], xsl(t, 0, d), AT.add).then_inc(
                    udve, 1
                )
                # z for the pool slice first so pool's mults can proceed
                for i in range(2):
                    lo, hi = cuts[i], cuts[i + 1]
                    dv.wait_ge(up[i], t + 1)  # pool U chunk (t) ready
                    if t >= 1:
                        dv.wait_ge(cm[i], t)  # pool C_i(t-1) read Z chunk
                    dv.tensor_scalar(
                        Z[:, lo:hi], usl(t, lo, hi), TH, TAU, AT.is_le, AT.mult
                    ).then_inc(zd[i], 1)
                dv.tensor_scalar(Z[:, 0:d], usl(t, 0, d), TH, TAU, AT.is_le, AT.mult)
                dv.tensor_tensor(Cc[:, 0:d], Z[:, 0:d], usl(t, 0, d), AT.mult)

        @block.gpsimd
        def _(pl):
            pl.memset(Cc[:, d:FD], 0.0)
            for t in range(T):
                pl.wait_ge(dx[t % XS], 16 * (t // XS + 1))  # X(t) loaded
                for i in range(2):
                    lo, hi = cuts[i], cuts[i + 1]
                    if t >= US:
                        pl.wait_ge(acs, t - 1)  # Sign_s(t-2) read U slot
                        pl.wait_ge(zd[i], t - 1)  # z_i(t-2) read U slot
                    pl.tensor_tensor(
                        usl(t, lo, hi), Cc[:, lo:hi], xsl(t, lo, hi), AT.add
                    ).then_inc(up[i], 1)
                for i in range(2):
                    lo, hi = cuts[i], cuts[i + 1]
                    pl.wait_ge(zd[i], t + 1)  # z_i(t) ready
                    pl.tensor_tensor(
                        Cc[:, lo:hi], Z[:, lo:hi], usl(t, lo, hi), AT.mult
                    ).then_inc(cm[i], 1)

    return nc


def _get_nc():
    if "nc" not in _cache:
        _cache["nc"] = _build_nc()
    return _cache["nc"]


def _shard_x(x):
    """x [T,B,C,H,W] fp32 -> list of 8 contiguous [T,128,2048] fp16 arrays."""
    xf = x.reshape(T, B, C, HWF)
    shards = []
    for i in range(N_CORES):
        xc = xf[:, i * B_PER : (i + 1) * B_PER]  # [T,4,C,1024]
        xc = xc.reshape(T, 2, 2, C, HWF).transpose(0, 1, 3, 2, 4)  # t,bp,c,bf,f
        shards.append(np.ascontiguousarray(xc).reshape(T, P, FD).astype(np.float16))
    return shards


def _unshard_o(per_core):
    """list of 8 [T,128,2048] int8 sign values -> [T,B,C,H,W] fp32 spikes."""
    outs = []
    for oc in per_core:
        oc = (oc == 1).astype(np.float32)
        oc = oc.reshape(T, 2, C, 2, HWF).transpose(0, 1, 3, 2, 4)  # t,bp,bf,c,f
        outs.append(oc.reshape(T, B_PER, C, H, W))
    return np.concatenate(outs, axis=1)


def kernel(x, w):
    global last_results
    x = np.ascontiguousarray(np.asarray(x), dtype=np.float32)
    w = np.tile(np.asarray(w, dtype=np.float32).reshape(64, 1), (2, 1))  # [128,1]

    nc = _get_nc()
    shards = _shard_x(x)
    in_maps = [{"x": shards[i], "w": w} for i in range(N_CORES)]
    last_results = run_bass_kernel_spmd(nc, in_maps, core_ids=list(range(N_CORES)))
    return _unshard_o([last_results.results[i]["o"] for i in range(N_CORES)])


# revision 9
# speedup vs baseline: 1.7651x; 1.0140x over previous
"""LIF spike kernel (T-step leaky integrate-and-fire recurrence) on 8 TRN2 cores.

Reference semantics (per element, thre = tanh(w[c])):
    u_t = TAU * u_{t-1} * (1 - o_{t-1}) + x_t
    o_t = (u_t - thre > 0) ? 1.0 : 0.0

This version trades bit-exactness for speed (harness gate is rel_err < 2e-2):
x and the carried state are fp16, which measures rel_err ~1.1e-2 on the
reference inputs and unlocks the DVE 2x/4x perf modes plus half the input DMA
traffic vs fp32.

Per step, with carry C = TAU*u*(u<=thre), the fp32 STT pair of the baseline is
replaced by a 3-op fp16 form (fp16 tensor_tensor runs at 2x and tensor_scalar
at 4x, while scalar_tensor_tensor is stuck at 1x):
    TT : U = C + X_t                 (tensor_tensor add)
    TS : z = (U <= thre) * TAU       (tensor_scalar; z in {0, TAU})
    TT : C = z * U                   (TAU = 2^-2 so the scale is exact)
Work split: DVE runs all 3 ops on cols [0,DCOL) plus the TS compare for the
rest; the gpsimd/Pool engine runs the TT add AND TT mult for cols [DCOL,2048)
(walrus only accepts add/mult TTs on Pool) in 2 chunks, ordered adds-first
then mults so the pool->DVE z-handoff pipelines instead of serializing.
ACT computes spikes as two Sign ops (one per slice, so each fires as soon as
its slice's U is ready): O = Sign(U - thre) -> int8 {-1,0,1}; the host maps
spike = (O == 1).  O stores are issued from the SP queue lagged LAG steps
behind the X loads so their semaphore waits never stall the load prefetch.

Sharding: B=32 split across 8 cores (4 each).  Per-core SBUF layout:
partition p = bp*64 + c (bp = batch pair, c = channel), free f = bf*1024 + hw,
with b = bp*2 + bf.  The host pre-transposes x to fp16 so each timestep is one
contiguous [128, 2048] DMA; spikes return as int8 and are mapped on host.
"""

import numpy as np

import concourse.bass as bass
import concourse.mybir as mybir
from concourse.bass_utils import run_bass_kernel_spmd

TAU = 0.25
T, B, C, H, W = 16, 32, 64, 32, 32
N_CORES = 8
B_PER = B // N_CORES  # 4
HWF = H * W  # 1024
P = 128  # partitions: 2 batch-pairs x 64 channels
FD = (B_PER // 2) * HWF  # 2048 free-dim elements per partition per step

DCOL = 1632  # DVE-only slice; cols [DCOL, FD) are computed by Pool (+DVE TS)
XS = 4  # X buffer slots
US = 2  # U slots
OS = 6  # O slots
LAG = 3  # store O(t-LAG) after loading X(t) on the SP queue

_cache = {}
last_results = None  # BassKernelResults of the most recent run (for test harness)


def _build_nc():
    nc = bass.Bass("TRN2", target_bir_lowering=False, debug=False, num_devices=N_CORES)
    f32 = mybir.dt.float32
    f16 = mybir.dt.float16
    i8 = mybir.dt.int8
    x_d = nc.dram_tensor("x", [T, P, FD], f16, kind="ExternalInput").ap()
    w_d = nc.dram_tensor("w", [P, 1], f32, kind="ExternalInput").ap()
    o_d = nc.dram_tensor("o", [T, P, FD], i8, kind="ExternalOutput").ap()

    AT = mybir.AluOpType
    AF = mybir.ActivationFunctionType

    d = DCOL
    s = FD - d
    c1 = d + (s * 45) // 100
    cuts = [d, c1, FD]

    X = nc.alloc_sbuf_tensor("Xb", [P, XS * FD], f16).ap()
    U = nc.alloc_sbuf_tensor("Ub", [P, US * FD], f16).ap()
    Cc = nc.alloc_sbuf_tensor("Cb", [P, FD], f16).ap()
    Z = nc.alloc_sbuf_tensor("Zb", [P, FD], f16).ap()
    O = nc.alloc_sbuf_tensor("Ob", [P, OS * FD], i8).ap()
    WT = nc.alloc_sbuf_tensor("WTb", [P, 1], f32).ap()
    NT = nc.alloc_sbuf_tensor("NTb", [P, 1], f32).ap()  # -tanh(w)
    TH = nc.alloc_sbuf_tensor("THb", [P, 1], f32).ap()  # +tanh(w)

    def xsl(t, lo, hi):
        return X[:, (t % XS) * FD + lo : (t % XS) * FD + hi]

    def usl(t, lo, hi):
        return U[:, (t % US) * FD + lo : (t % US) * FD + hi]

    def osl(t, lo=0, hi=FD):
        return O[:, (t % OS) * FD + lo : (t % OS) * FD + hi]

    import contextlib

    with contextlib.ExitStack() as st:
        block = st.enter_context(nc.Block())
        udve = st.enter_context(nc.semaphore("udve"))  # t+1 after TT-U_d(t)
        act = st.enter_context(nc.semaphore("act"))  # 1 + (t+1) after Sign_d(t)
        acs = st.enter_context(nc.semaphore("acs"))  # t+1 after Sign_s(t)
        dw = st.enter_context(nc.semaphore("dw"))
        up = [st.enter_context(nc.semaphore(f"up{i}")) for i in range(2)]
        zd = [st.enter_context(nc.semaphore(f"zd{i}")) for i in range(2)]
        cm = [st.enter_context(nc.semaphore(f"cm{i}")) for i in range(2)]
        dx = [st.enter_context(nc.semaphore(f"dx{i}")) for i in range(XS)]
        do = [st.enter_context(nc.semaphore(f"do{i}")) for i in range(OS)]

        @block.sync
        def _(sp):
            for t in range(T):
                if t >= XS:
                    # X slot's previous tenant (t-XS) consumed by both engines
                    sp.wait_ge(udve, t - XS + 1)
                    sp.wait_ge(up[1], t - XS + 1)
                sp.dma_start(out=xsl(t, 0, FD), in_=x_d[t]).then_inc(dx[t % XS], 16)
                if t == 0:
                    sp.dma_start(out=WT, in_=w_d).then_inc(dw, 16)
                if t >= LAG:
                    tt = t - LAG
                    sp.wait_ge(act, tt + 2)  # Sign_d(tt) done
                    sp.wait_ge(acs, tt + 1)  # Sign_s(tt) done
                    sp.dma_start(out=o_d[tt], in_=osl(tt)).then_inc(do[tt % OS], 16)
            for t in range(T - LAG, T):
                sp.wait_ge(act, t + 2)
                sp.wait_ge(acs, t + 1)
                sp.dma_start(out=o_d[t], in_=osl(t)).then_inc(do[t % OS], 16)
            for i in range(OS):
                n_dmas = len([t for t in range(T) if t % OS == i])
                sp.wait_ge(do[i], 16 * n_dmas)

        @block.scalar
        def _(ac):
            ac.wait_ge(dw, 16)
            ac.activation(NT, WT, AF.Tanh, scale=-1.0)  # tanh odd: -tanh(w)
            ac.activation(TH, WT, AF.Tanh).then_inc(act, 1)
            ac.drain()
            for t in range(T):
                ac.wait_ge(udve, t + 1)  # U[:, :d](t) ready
                if t >= OS:
                    ac.wait_ge(do[t % OS], 16 * (t // OS))  # O slot drained
                ac.activation(osl(t, 0, d), usl(t, 0, d), AF.Sign, bias=NT).then_inc(
                    act, 1
                )
                ac.wait_ge(up[1], t + 1)  # U[:, d:](t) ready
                ac.activation(osl(t, d, FD), usl(t, d, FD), AF.Sign, bias=NT).then_inc(
                    acs, 1
                )

        @block.vector
        def _(dv):
            dv.memset(Cc[:, 0:d], 0.0)
            dv.wait_ge(act, 1)  # thre ready
            for t in range(T):
                dv.wait_ge(dx[t % XS], 16 * (t // XS + 1))  # X(t) loaded
                if t >= US:
                    dv.wait_ge(act, t)  # Sign_d(t-2) read its U slot
                dv.tensor_tensor(usl(t, 0, d), Cc[:, 0:d], xsl(t, 0, d), AT.add).then_inc(
                    udve, 1
                )
                # one merged z over [0, c1) (d-slice + pool chunk0), then chunk1
                dv.wait_ge(up[0], t + 1)  # pool U chunk0 (t) ready
                if t >= 1:
                    dv.wait_ge(cm[0], t)  # pool C_0(t-1) read its Z chunk
                dv.tensor_scalar(
                    Z[:, 0:c1], usl(t, 0, c1), TH, TAU, AT.is_le, AT.mult
                ).then_inc(zd[0], 1)
                dv.wait_ge(up[1], t + 1)  # pool U chunk1 (t) ready
                if t >= 1:
                    dv.wait_ge(cm[1], t)  # pool C_1(t-1) read its Z chunk
                dv.tensor_scalar(
                    Z[:, c1:FD], usl(t, c1, FD), TH, TAU, AT.is_le, AT.mult
                ).then_inc(zd[1], 1)
                dv.tensor_tensor(Cc[:, 0:d], Z[:, 0:d], usl(t, 0, d), AT.mult)

        @block.gpsimd
        def _(pl):
            pl.memset(Cc[:, d:FD], 0.0)
            for t in range(T):
                pl.wait_ge(dx[t % XS], 16 * (t // XS + 1))  # X(t) loaded
                for i in range(2):
                    lo, hi = cuts[i], cuts[i + 1]
                    if t >= US:
                        pl.wait_ge(acs, t - 1)  # Sign_s(t-2) read U slot
                        pl.wait_ge(zd[i], t - 1)  # z_i(t-2) read U slot
                    pl.tensor_tensor(
                        usl(t, lo, hi), Cc[:, lo:hi], xsl(t, lo, hi), AT.add
                    ).then_inc(up[i], 1)
                for i in range(2):
                    lo, hi = cuts[i], cuts[i + 1]
                    pl.wait_ge(zd[i], t + 1)  # z_i(t) ready
                    pl.tensor_tensor(
                        Cc[:, lo:hi], Z[:, lo:hi], usl(t, lo, hi), AT.mult
                    ).then_inc(cm[i], 1)

    return nc


def _get_nc():
    if "nc" not in _cache:
        _cache["nc"] = _build_nc()
    return _cache["nc"]


def _shard_x(x):
    """x [T,B,C,H,W] fp32 -> list of 8 contiguous [T,128,2048] fp16 arrays."""
    xf = x.reshape(T, B, C, HWF)
    shards = []
    for i in range(N_CORES):
        xc = xf[:, i * B_PER : (i + 1) * B_PER]  # [T,4,C,1024]
        xc = xc.reshape(T, 2, 2, C, HWF).transpose(0, 1, 3, 2, 4)  # t,bp,c,bf,f
        shards.append(np.ascontiguousarray(xc).reshape(T, P, FD).astype(np.float16))
    return shards


def _unshard_o(per_core):
    """list of 8 [T,128,2048] int8 sign values -> [T,B,C,H,W] fp32 spikes."""
    outs = []
    for oc in per_core:
        oc = (oc == 1).astype(np.float32)
        oc = oc.reshape(T, 2, C, 2, HWF).transpose(0, 1, 3, 2, 4)  # t,bp,bf,c,f
        outs.append(oc.reshape(T, B_PER, C, H, W))
    return np.concatenate(outs, axis=1)


def kernel(x, w):
    global last_results
    x = np.ascontiguousarray(np.asarray(x), dtype=np.float32)
    w = np.tile(np.asarray(w, dtype=np.float32).reshape(64, 1), (2, 1))  # [128,1]

    nc = _get_nc()
    shards = _shard_x(x)
    in_maps = [{"x": shards[i], "w": w} for i in range(N_CORES)]
    last_results = run_bass_kernel_spmd(nc, in_maps, core_ids=list(range(N_CORES)))
    return _unshard_o([last_results.results[i]["o"] for i in range(N_CORES)])


# revision 11
# speedup vs baseline: 1.7786x; 1.0077x over previous
"""LIF spike kernel (T-step leaky integrate-and-fire recurrence) on 8 TRN2 cores.

Reference semantics (per element, thre = tanh(w[c])):
    u_t = TAU * u_{t-1} * (1 - o_{t-1}) + x_t
    o_t = (u_t - thre > 0) ? 1.0 : 0.0

This version trades bit-exactness for speed (harness gate is rel_err < 2e-2):
x and the carried state are fp16, which measures rel_err ~1.1e-2 on the
reference inputs and unlocks the DVE 2x/4x perf modes plus half the input DMA
traffic vs fp32.

Per step, with carry C = TAU*u*(u<=thre), the fp32 STT pair of the baseline is
replaced by a 3-op fp16 form (fp16 tensor_tensor runs at 2x and tensor_scalar
at 4x, while scalar_tensor_tensor is stuck at 1x):
    TT : U = C + X_t                 (tensor_tensor add)
    TS : z = (U <= thre) * TAU       (tensor_scalar; z in {0, TAU})
    TT : C = z * U                   (TAU = 2^-2 so the scale is exact)
Work split: DVE runs TT-add/TS/TT-mult on cols [0,DCOL) plus the TS compares
for everything (one TS over [0,c1), one over [c1,FD)); the gpsimd/Pool engine
runs the TT add AND TT mult for cols [DCOL,2048) (walrus only accepts add/mult
TTs on Pool) in 2 chunks, ordered adds-first then mults so the pool->DVE z
handoff pipelines.  ACT computes spikes as two Sign ops (one per slice):
O = Sign(U - thre) -> int8 {-1,0,1}; the host maps spike = (O == 1).
O stores are issued from the SP queue lagged LAG steps behind the X loads so
their semaphore waits never stall the load prefetch.  Semaphores are merged
(pool U/C share pc[i]; both Signs share act) to minimize per-step wait count
on the DVE sequencer.

Sharding: B=32 split across 8 cores (4 each).  Per-core SBUF layout:
partition p = bp*64 + c (bp = batch pair, c = channel), free f = bf*1024 + hw,
with b = bp*2 + bf.  The host pre-transposes x to fp16 so each timestep is one
contiguous [128, 2048] DMA; spikes return as int8 and are mapped on host.
"""

import numpy as np

import concourse.bass as bass
import concourse.mybir as mybir
from concourse.bass_utils import run_bass_kernel_spmd

TAU = 0.25
T, B, C, H, W = 16, 32, 64, 32, 32
N_CORES = 8
B_PER = B // N_CORES  # 4
HWF = H * W  # 1024
P = 128  # partitions: 2 batch-pairs x 64 channels
FD = (B_PER // 2) * HWF  # 2048 free-dim elements per partition per step

DCOL = 1608  # DVE-only slice; cols [DCOL, FD) are computed by Pool (+DVE TS)
XS = 4  # X buffer slots
US = 2  # U slots
OS = 6  # O slots
LAG = 3  # store O(t-LAG) after loading X(t) on the SP queue

_cache = {}
last_results = None  # BassKernelResults of the most recent run (for test harness)


def _build_nc():
    nc = bass.Bass("TRN2", target_bir_lowering=False, debug=False, num_devices=N_CORES)
    f32 = mybir.dt.float32
    f16 = mybir.dt.float16
    i8 = mybir.dt.int8
    x_d = nc.dram_tensor("x", [T, P, FD], f16, kind="ExternalInput").ap()
    w_d = nc.dram_tensor("w", [P, 1], f32, kind="ExternalInput").ap()
    o_d = nc.dram_tensor("o", [T, P, FD], i8, kind="ExternalOutput").ap()

    AT = mybir.AluOpType
    AF = mybir.ActivationFunctionType

    d = DCOL
    s = FD - d
    c1 = d + (s * 45) // 100
    cuts = [d, c1, FD]

    X = nc.alloc_sbuf_tensor("Xb", [P, XS * FD], f16).ap()
    U = nc.alloc_sbuf_tensor("Ub", [P, US * FD], f16).ap()
    Cc = nc.alloc_sbuf_tensor("Cb", [P, FD], f16).ap()
    Z = nc.alloc_sbuf_tensor("Zb", [P, FD], f16).ap()
    O = nc.alloc_sbuf_tensor("Ob", [P, OS * FD], i8).ap()
    WT = nc.alloc_sbuf_tensor("WTb", [P, 1], f32).ap()
    NT = nc.alloc_sbuf_tensor("NTb", [P, 1], f32).ap()  # -tanh(w)
    TH = nc.alloc_sbuf_tensor("THb", [P, 1], f32).ap()  # +tanh(w)

    def xsl(t, lo, hi):
        return X[:, (t % XS) * FD + lo : (t % XS) * FD + hi]

    def usl(t, lo, hi):
        return U[:, (t % US) * FD + lo : (t % US) * FD + hi]

    def osl(t, lo=0, hi=FD):
        return O[:, (t % OS) * FD + lo : (t % OS) * FD + hi]

    import contextlib

    with contextlib.ExitStack() as st:
        block = st.enter_context(nc.Block())
        udve = st.enter_context(nc.semaphore("udve"))  # t+1 after TT-U_d(t)
        # act: +1 tanh, +1 Sign_d(t), +1 Sign_s(t)  ->  2t+3 after step t
        act = st.enter_context(nc.semaphore("act"))
        dw = st.enter_context(nc.semaphore("dw"))
        # pc[i]: pool chunk i; U_i(t) -> 2t+1, C_i(t) -> 2t+2
        pc = [st.enter_context(nc.semaphore(f"pc{i}")) for i in range(2)]
        zd = [st.enter_context(nc.semaphore(f"zd{i}")) for i in range(2)]
        dx = [st.enter_context(nc.semaphore(f"dx{i}")) for i in range(XS)]
        do = [st.enter_context(nc.semaphore(f"do{i}")) for i in range(OS)]

        @block.sync
        def _(sp):
            for t in range(T):
                if t >= XS:
                    # X slot's previous tenant (t-XS) consumed by both engines
                    sp.wait_ge(udve, t - XS + 1)
                    sp.wait_ge(pc[1], 2 * (t - XS) + 1)
                sp.dma_start(out=xsl(t, 0, FD), in_=x_d[t]).then_inc(dx[t % XS], 16)
                if t == 0:
                    sp.dma_start(out=WT, in_=w_d).then_inc(dw, 16)
                if t >= LAG:
                    tt = t - LAG
                    sp.wait_ge(act, 2 * tt + 3)  # both Signs(tt) done
                    sp.dma_start(out=o_d[tt], in_=osl(tt)).then_inc(do[tt % OS], 16)
            for t in range(T - LAG, T):
                sp.wait_ge(act, 2 * t + 3)
                sp.dma_start(out=o_d[t], in_=osl(t)).then_inc(do[t % OS], 16)
            for i in range(OS):
                n_dmas = len([t for t in range(T) if t % OS == i])
                sp.wait_ge(do[i], 16 * n_dmas)

        @block.scalar
        def _(ac):
            ac.wait_ge(dw, 16)
            ac.activation(NT, WT, AF.Tanh, scale=-1.0)  # tanh odd: -tanh(w)
            ac.activation(TH, WT, AF.Tanh).then_inc(act, 1)
            ac.drain()
            for t in range(T):
                ac.wait_ge(udve, t + 1)  # U[:, :d](t) ready
                if t >= OS:
                    ac.wait_ge(do[t % OS], 16 * (t // OS))  # O slot drained
                ac.activation(osl(t, 0, d), usl(t, 0, d), AF.Sign, bias=NT).then_inc(
                    act, 1
                )
                ac.wait_ge(pc[1], 2 * t + 1)  # U[:, d:](t) ready
                ac.activation(osl(t, d, FD), usl(t, d, FD), AF.Sign, bias=NT).then_inc(
                    act, 1
                )

        @block.vector
        def _(dv):
            dv.memset(Cc[:, 0:d], 0.0)
            dv.wait_ge(act, 1)  # thre ready
            for t in range(T):
                dv.wait_ge(dx[t % XS], 16 * (t // XS + 1))  # X(t) loaded
                if t >= US:
                    dv.wait_ge(act, 2 * (t - US) + 2)  # Sign_d(t-US) read U slot
                dv.tensor_tensor(usl(t, 0, d), Cc[:, 0:d], xsl(t, 0, d), AT.add).then_inc(
                    udve, 1
                )
                # merged z over [0, c1): d-slice + pool chunk0.  pc[i] >= 2t+1
                # means pool U_i(t) is done, which also implies C_i(t-1) has
                # read its Z chunk (pool program order), so one wait covers
                # both the RAW (U ready) and WAR (Z reusable) hazards.
                dv.wait_ge(pc[0], 2 * t + 1)
                dv.tensor_scalar(
                    Z[:, 0:c1], usl(t, 0, c1), TH, TAU, AT.is_le, AT.mult
                ).then_inc(zd[0], 1)
                dv.wait_ge(pc[1], 2 * t + 1)
                dv.tensor_scalar(
                    Z[:, c1:FD], usl(t, c1, FD), TH, TAU, AT.is_le, AT.mult
                ).then_inc(zd[1], 1)
                dv.tensor_tensor(Cc[:, 0:d], Z[:, 0:d], usl(t, 0, d), AT.mult)

        @block.gpsimd
        def _(pl):
            pl.memset(Cc[:, d:FD], 0.0)
            for t in range(T):
                pl.wait_ge(dx[t % XS], 16 * (t // XS + 1))  # X(t) loaded
                for i in range(2):
                    lo, hi = cuts[i], cuts[i + 1]
                    if t >= US:
                        pl.wait_ge(act, 2 * (t - US) + 3)  # Sign_s(t-US) read U
                        pl.wait_ge(zd[i], t - US + 1)  # z_i(t-US) read U slot
                    pl.tensor_tensor(
                        usl(t, lo, hi), Cc[:, lo:hi], xsl(t, lo, hi), AT.add
                    ).then_inc(pc[i], 1)
                for i in range(2):
                    lo, hi = cuts[i], cuts[i + 1]
                    pl.wait_ge(zd[i], t + 1)  # z_i(t) ready
                    pl.tensor_tensor(
                        Cc[:, lo:hi], Z[:, lo:hi], usl(t, lo, hi), AT.mult
                    ).then_inc(pc[i], 1)

    return nc


def _get_nc():
    if "nc" not in _cache:
        _cache["nc"] = _build_nc()
    return _cache["nc"]


def _shard_x(x):
    """x [T,B,C,H,W] fp32 -> list of 8 contiguous [T,128,2048] fp16 arrays."""
    xf = x.reshape(T, B, C, HWF)
    shards = []
    for i in range(N_CORES):
        xc = xf[:, i * B_PER : (i + 1) * B_PER]  # [T,4,C,1024]
        xc = xc.reshape(T, 2, 2, C, HWF).transpose(0, 1, 3, 2, 4)  # t,bp,c,bf,f
        shards.append(np.ascontiguousarray(xc).reshape(T, P, FD).astype(np.float16))
    return shards


def _unshard_o(per_core):
    """list of 8 [T,128,2048] int8 sign values -> [T,B,C,H,W] fp32 spikes."""
    outs = []
    for oc in per_core:
        oc = (oc == 1).astype(np.float32)
        oc = oc.reshape(T, 2, C, 2, HWF).transpose(0, 1, 3, 2, 4)  # t,bp,bf,c,f
        outs.append(oc.reshape(T, B_PER, C, H, W))
    return np.concatenate(outs, axis=1)


def kernel(x, w):
    global last_results
    x = np.ascontiguousarray(np.asarray(x), dtype=np.float32)
    w = np.tile(np.asarray(w, dtype=np.float32).reshape(64, 1), (2, 1))  # [128,1]

    nc = _get_nc()
    shards = _shard_x(x)
    in_maps = [{"x": shards[i], "w": w} for i in range(N_CORES)]
    last_results = run_bass_kernel_spmd(nc, in_maps, core_ids=list(range(N_CORES)))
    return _unshard_o([last_results.results[i]["o"] for i in range(N_CORES)])


# revision 12
# speedup vs baseline: 1.7844x; 1.0032x over previous
"""LIF spike kernel (T-step leaky integrate-and-fire recurrence) on 8 TRN2 cores.

Reference semantics (per element, thre = tanh(w[c])):
    u_t = TAU * u_{t-1} * (1 - o_{t-1}) + x_t
    o_t = (u_t - thre > 0) ? 1.0 : 0.0

This version trades bit-exactness for speed (harness gate is rel_err < 2e-2):
x and the carried state are fp16, which measures rel_err ~1.1e-2 on the
reference inputs and unlocks the DVE 2x/4x perf modes plus half the input DMA
traffic vs fp32.

Per step, with carry C = TAU*u*(u<=thre), the fp32 STT pair of the baseline is
replaced by a 3-op fp16 form (fp16 tensor_tensor runs at 2x and tensor_scalar
at 4x, while scalar_tensor_tensor is stuck at 1x):
    TT : U = C + X_t                 (tensor_tensor add)
    TS : z = (U <= thre) * TAU       (tensor_scalar; z in {0, TAU})
    TT : C = z * U                   (TAU = 2^-2 so the scale is exact)
Work split: DVE runs TT-add/TS/TT-mult on cols [0,DCOL) plus the TS compares
for everything (one TS over [0,c1), one over [c1,FD)); the gpsimd/Pool engine
runs the TT add AND TT mult for cols [DCOL,2048) (walrus only accepts add/mult
TTs on Pool) in 2 chunks, ordered adds-first then mults so the pool->DVE z
handoff pipelines.  ACT computes spikes as two Sign ops (one per slice):
O = Sign(U - thre) -> int8 {-1,0,1}; the host maps spike = (O == 1).
O stores are issued from the SP queue lagged LAG steps behind the X loads so
their semaphore waits never stall the load prefetch.  Semaphores are merged
(pool U/C share pc[i]; both Signs share act) to minimize per-step wait count
on the DVE sequencer.

Sharding: B=32 split across 8 cores (4 each).  Per-core SBUF layout:
partition p = bp*64 + c (bp = batch pair, c = channel), free f = bf*1024 + hw,
with b = bp*2 + bf.  The host pre-transposes x to fp16 so each timestep is one
contiguous [128, 2048] DMA; spikes return as int8 and are mapped on host.
"""

import numpy as np

import concourse.bass as bass
import concourse.mybir as mybir
from concourse.bass_utils import run_bass_kernel_spmd

TAU = 0.25
T, B, C, H, W = 16, 32, 64, 32, 32
N_CORES = 8
B_PER = B // N_CORES  # 4
HWF = H * W  # 1024
P = 128  # partitions: 2 batch-pairs x 64 channels
FD = (B_PER // 2) * HWF  # 2048 free-dim elements per partition per step

DCOL = 1608  # DVE-only slice; cols [DCOL, FD) are computed by Pool (+DVE TS)
XS = 4  # X buffer slots
US = 2  # U slots
OS = 16  # O slots (one per step: no store-slot recycling waits)
LAG = 3  # store O(t-LAG) after loading X(t) on the SP queue

_cache = {}
last_results = None  # BassKernelResults of the most recent run (for test harness)


def _build_nc():
    nc = bass.Bass("TRN2", target_bir_lowering=False, debug=False, num_devices=N_CORES)
    f32 = mybir.dt.float32
    f16 = mybir.dt.float16
    i8 = mybir.dt.int8
    x_d = nc.dram_tensor("x", [T, P, FD], f16, kind="ExternalInput").ap()
    w_d = nc.dram_tensor("w", [P, 1], f32, kind="ExternalInput").ap()
    o_d = nc.dram_tensor("o", [T, P, FD], i8, kind="ExternalOutput").ap()

    AT = mybir.AluOpType
    AF = mybir.ActivationFunctionType

    d = DCOL
    s = FD - d
    c1 = d + (s * 45) // 100
    cuts = [d, c1, FD]

    X = nc.alloc_sbuf_tensor("Xb", [P, XS * FD], f16).ap()
    U = nc.alloc_sbuf_tensor("Ub", [P, US * FD], f16).ap()
    Cc = nc.alloc_sbuf_tensor("Cb", [P, FD], f16).ap()
    Z = nc.alloc_sbuf_tensor("Zb", [P, FD], f16).ap()
    O = nc.alloc_sbuf_tensor("Ob", [P, OS * FD], i8).ap()
    WT = nc.alloc_sbuf_tensor("WTb", [P, 1], f32).ap()
    NT = nc.alloc_sbuf_tensor("NTb", [P, 1], f32).ap()  # -tanh(w)
    TH = nc.alloc_sbuf_tensor("THb", [P, 1], f32).ap()  # +tanh(w)

    def xsl(t, lo, hi):
        return X[:, (t % XS) * FD + lo : (t % XS) * FD + hi]

    def usl(t, lo, hi):
        return U[:, (t % US) * FD + lo : (t % US) * FD + hi]

    def osl(t, lo=0, hi=FD):
        return O[:, (t % OS) * FD + lo : (t % OS) * FD + hi]

    import contextlib

    with contextlib.ExitStack() as st:
        block = st.enter_context(nc.Block())
        udve = st.enter_context(nc.semaphore("udve"))  # t+1 after TT-U_d(t)
        # act: +1 tanh, +1 Sign_d(t), +1 Sign_s(t)  ->  2t+3 after step t
        act = st.enter_context(nc.semaphore("act"))
        dw = st.enter_context(nc.semaphore("dw"))
        # pc[i]: pool chunk i; U_i(t) -> 2t+1, C_i(t) -> 2t+2
        pc = [st.enter_context(nc.semaphore(f"pc{i}")) for i in range(2)]
        zd = [st.enter_context(nc.semaphore(f"zd{i}")) for i in range(2)]
        dx = [st.enter_context(nc.semaphore(f"dx{i}")) for i in range(XS)]
        do = [st.enter_context(nc.semaphore(f"do{i}")) for i in range(OS)]

        @block.sync
        def _(sp):
            for t in range(T):
                if t >= XS:
                    # X slot's previous tenant (t-XS) consumed by both engines
                    sp.wait_ge(udve, t - XS + 1)
                    sp.wait_ge(pc[1], 2 * (t - XS) + 1)
                sp.dma_start(out=xsl(t, 0, FD), in_=x_d[t]).then_inc(dx[t % XS], 16)
                if t == 0:
                    sp.dma_start(out=WT, in_=w_d).then_inc(dw, 16)
                if t >= LAG:
                    tt = t - LAG
                    sp.wait_ge(act, 2 * tt + 3)  # both Signs(tt) done
                    sp.dma_start(out=o_d[tt], in_=osl(tt)).then_inc(do[tt % OS], 16)
            for t in range(T - LAG, T):
                sp.wait_ge(act, 2 * t + 3)
                sp.dma_start(out=o_d[t], in_=osl(t)).then_inc(do[t % OS], 16)
            for i in range(OS):
                n_dmas = len([t for t in range(T) if t % OS == i])
                sp.wait_ge(do[i], 16 * n_dmas)

        @block.scalar
        def _(ac):
            ac.wait_ge(dw, 16)
            ac.activation(NT, WT, AF.Tanh, scale=-1.0)  # tanh odd: -tanh(w)
            ac.activation(TH, WT, AF.Tanh).then_inc(act, 1)
            ac.drain()
            for t in range(T):
                ac.wait_ge(udve, t + 1)  # U[:, :d](t) ready
                if t >= OS:
                    ac.wait_ge(do[t % OS], 16 * (t // OS))  # O slot drained
                ac.activation(osl(t, 0, d), usl(t, 0, d), AF.Sign, bias=NT).then_inc(
                    act, 1
                )
                ac.wait_ge(pc[1], 2 * t + 1)  # U[:, d:](t) ready
                ac.activation(osl(t, d, FD), usl(t, d, FD), AF.Sign, bias=NT).then_inc(
                    act, 1
                )

        @block.vector
        def _(dv):
            dv.memset(Cc[:, 0:d], 0.0)
            dv.wait_ge(act, 1)  # thre ready
            for t in range(T):
                dv.wait_ge(dx[t % XS], 16 * (t // XS + 1))  # X(t) loaded
                if t >= US:
                    dv.wait_ge(act, 2 * (t - US) + 2)  # Sign_d(t-US) read U slot
                dv.tensor_tensor(usl(t, 0, d), Cc[:, 0:d], xsl(t, 0, d), AT.add).then_inc(
                    udve, 1
                )
                # merged z over [0, c1): d-slice + pool chunk0.  pc[i] >= 2t+1
                # means pool U_i(t) is done, which also implies C_i(t-1) has
                # read its Z chunk (pool program order), so one wait covers
                # both the RAW (U ready) and WAR (Z reusable) hazards.
                dv.wait_ge(pc[0], 2 * t + 1)
                dv.tensor_scalar(
                    Z[:, 0:c1], usl(t, 0, c1), TH, TAU, AT.is_le, AT.mult
                ).then_inc(zd[0], 1)
                dv.wait_ge(pc[1], 2 * t + 1)
                dv.tensor_scalar(
                    Z[:, c1:FD], usl(t, c1, FD), TH, TAU, AT.is_le, AT.mult
                ).then_inc(zd[1], 1)
                dv.tensor_tensor(Cc[:, 0:d], Z[:, 0:d], usl(t, 0, d), AT.mult)

        @block.gpsimd
        def _(pl):
            pl.memset(Cc[:, d:FD], 0.0)
            for t in range(T):
                pl.wait_ge(dx[t % XS], 16 * (t // XS + 1))  # X(t) loaded
                for i in range(2):
                    lo, hi = cuts[i], cuts[i + 1]
                    if t >= US:
                        pl.wait_ge(act, 2 * (t - US) + 3)  # Sign_s(t-US) read U
                        pl.wait_ge(zd[i], t - US + 1)  # z_i(t-US) read U slot
                    pl.tensor_tensor(
                        usl(t, lo, hi), Cc[:, lo:hi], xsl(t, lo, hi), AT.add
                    ).then_inc(pc[i], 1)
                for i in range(2):
                    lo, hi = cuts[i], cuts[i + 1]
                    pl.wait_ge(zd[i], t + 1)  # z_i(t) ready
                    pl.tensor_tensor(
                        Cc[:, lo:hi], Z[:, lo:hi], usl(t, lo, hi), AT.mult
                    ).then_inc(pc[i], 1)

    return nc


def _get_nc():
    if "nc" not in _cache:
        _cache["nc"] = _build_nc()
    return _cache["nc"]


def _shard_x(x):
    """x [T,B,C,H,W] fp32 -> list of 8 contiguous [T,128,2048] fp16 arrays."""
    xf = x.reshape(T, B, C, HWF)
    shards = []
    for i in range(N_CORES):
        xc = xf[:, i * B_PER : (i + 1) * B_PER]  # [T,4,C,1024]
        xc = xc.reshape(T, 2, 2, C, HWF).transpose(0, 1, 3, 2, 4)  # t,bp,c,bf,f
        shards.append(np.ascontiguousarray(xc).reshape(T, P, FD).astype(np.float16))
    return shards


def _unshard_o(per_core):
    """list of 8 [T,128,2048] int8 sign values -> [T,B,C,H,W] fp32 spikes."""
    outs = []
    for oc in per_core:
        oc = (oc == 1).astype(np.float32)
        oc = oc.reshape(T, 2, C, 2, HWF).transpose(0, 1, 3, 2, 4)  # t,bp,bf,c,f
        outs.append(oc.reshape(T, B_PER, C, H, W))
    return np.concatenate(outs, axis=1)


def kernel(x, w):
    global last_results
    x = np.ascontiguousarray(np.asarray(x), dtype=np.float32)
    w = np.tile(np.asarray(w, dtype=np.float32).reshape(64, 1), (2, 1))  # [128,1]

    nc = _get_nc()
    shards = _shard_x(x)
    in_maps = [{"x": shards[i], "w": w} for i in range(N_CORES)]
    last_results = run_bass_kernel_spmd(nc, in_maps, core_ids=list(range(N_CORES)))
    return _unshard_o([last_results.results[i]["o"] for i in range(N_CORES)])


# revision 16
# speedup vs baseline: 1.8013x; 1.0095x over previous
"""LIF spike kernel (T-step leaky integrate-and-fire recurrence) on 8 TRN2 cores.

Reference semantics (per element, thre = tanh(w[c])):
    u_t = TAU * u_{t-1} * (1 - o_{t-1}) + x_t
    o_t = (u_t - thre > 0) ? 1.0 : 0.0

This version trades bit-exactness for speed (harness gate is rel_err < 2e-2):
x and the carried state are fp16, which measures rel_err ~1.1e-2 on the
reference inputs and unlocks the DVE 2x/4x perf modes plus half the input DMA
traffic vs fp32.

Per step, with carry C = TAU*u*(u<=thre), the fp32 STT pair of the baseline is
replaced by a 3-op fp16 form (fp16 tensor_tensor runs at 2x and tensor_scalar
at 4x, while scalar_tensor_tensor is stuck at 1x):
    TT : U = C + X_t                 (tensor_tensor add)
    TS : z = (U <= thre) * TAU       (tensor_scalar; z in {0, TAU})
    TT : C = z * U                   (TAU = 2^-2 so the scale is exact)
Work split: DVE runs TT-add/TS/TT-mult on cols [0,DCOL) plus the TS compares
for everything (one TS over [0,c1), one over [c1,FD)); the gpsimd/Pool engine
runs the TT add AND TT mult for cols [DCOL,2048) (walrus only accepts add/mult
TTs on Pool) in 2 chunks, ordered adds-first then mults so the pool->DVE z
handoff pipelines.  ACT computes spikes as two Sign ops (one per slice):
O = Sign(U - thre) -> int8 {-1,0,1}; the host maps spike = (O == 1).
O stores are issued from the SP queue lagged LAG steps behind the X loads so
their semaphore waits never stall the load prefetch.  Semaphores are merged
(pool U/C share pc[i]; both Signs share act) to minimize per-step wait count
on the DVE sequencer.

Sharding: B=32 split across 8 cores (4 each).  Per-core SBUF layout:
partition p = bp*64 + c (bp = batch pair, c = channel), free f = bf*1024 + hw,
with b = bp*2 + bf.  The host pre-transposes x to fp16 so each timestep is one
contiguous [128, 2048] DMA; spikes return as int8 and are mapped on host.
"""

import numpy as np

import concourse.bass as bass
import concourse.mybir as mybir
from concourse.bass_utils import run_bass_kernel_spmd

TAU = 0.25
T, B, C, H, W = 16, 32, 64, 32, 32
N_CORES = 8
B_PER = B // N_CORES  # 4
HWF = H * W  # 1024
P = 128  # partitions: 2 batch-pairs x 64 channels
FD = (B_PER // 2) * HWF  # 2048 free-dim elements per partition per step

DCOL = 1608  # DVE-only slice; cols [DCOL, FD) are computed by Pool (+DVE TS)
XS = 4  # X buffer slots
US = 2  # U slots
OS = 16  # O slots (one per step: no store-slot recycling waits)
LAG = 3  # store O(t-LAG) after loading X(t) on the SP queue

_cache = {}
last_results = None  # BassKernelResults of the most recent run (for test harness)


def _build_nc():
    nc = bass.Bass("TRN2", target_bir_lowering=False, debug=False, num_devices=N_CORES)
    f32 = mybir.dt.float32
    f16 = mybir.dt.float16
    i8 = mybir.dt.int8
    x_d = nc.dram_tensor("x", [T, P, FD], f16, kind="ExternalInput").ap()
    w_d = nc.dram_tensor("w", [P, 1], f32, kind="ExternalInput").ap()
    o_d = nc.dram_tensor("o", [T, P, FD], i8, kind="ExternalOutput").ap()

    AT = mybir.AluOpType
    AF = mybir.ActivationFunctionType

    d = DCOL
    s = FD - d
    c1 = d + (s * 45) // 100
    cuts = [d, c1, FD]

    X = nc.alloc_sbuf_tensor("Xb", [P, XS * FD], f16).ap()
    U = nc.alloc_sbuf_tensor("Ub", [P, US * FD], f16).ap()
    Cc = nc.alloc_sbuf_tensor("Cb", [P, FD], f16).ap()
    Z = nc.alloc_sbuf_tensor("Zb", [P, FD], f16).ap()
    O = nc.alloc_sbuf_tensor("Ob", [P, OS * FD], i8).ap()
    WT = nc.alloc_sbuf_tensor("WTb", [P, 1], f32).ap()
    NT = nc.alloc_sbuf_tensor("NTb", [P, 1], f32).ap()  # -tanh(w)
    TH = nc.alloc_sbuf_tensor("THb", [P, 1], f32).ap()  # +tanh(w)

    def xsl(t, lo, hi):
        return X[:, (t % XS) * FD + lo : (t % XS) * FD + hi]

    def usl(t, lo, hi):
        return U[:, (t % US) * FD + lo : (t % US) * FD + hi]

    def osl(t, lo=0, hi=FD):
        return O[:, (t % OS) * FD + lo : (t % OS) * FD + hi]

    import contextlib

    with contextlib.ExitStack() as st:
        block = st.enter_context(nc.Block())
        udve = st.enter_context(nc.semaphore("udve"))  # t+1 after TT-U_d(t)
        # act: +1 tanh, +1 Sign_d(t), +1 Sign_s(t)  ->  2t+3 after step t
        act = st.enter_context(nc.semaphore("act"))
        dw = st.enter_context(nc.semaphore("dw"))
        # pc[i]: pool chunk i; U_i(t) -> 2t+1, C_i(t) -> 2t+2
        pc = [st.enter_context(nc.semaphore(f"pc{i}")) for i in range(2)]
        zd = [st.enter_context(nc.semaphore(f"zd{i}")) for i in range(2)]
        dx = [st.enter_context(nc.semaphore(f"dx{i}")) for i in range(XS)]
        do = [st.enter_context(nc.semaphore(f"do{i}")) for i in range(OS)]

        @block.sync
        def _(sp):
            for t in range(T):
                if t >= XS:
                    # X slot's previous tenant (t-XS) consumed by both engines
                    sp.wait_ge(udve, t - XS + 1)
                    sp.wait_ge(pc[1], 2 * (t - XS) + 1)
                sp.dma_start(out=xsl(t, 0, FD), in_=x_d[t]).then_inc(dx[t % XS], 16)
                if t == 0:
                    sp.dma_start(out=WT, in_=w_d).then_inc(dw, 16)
                if t >= LAG:
                    tt = t - LAG
                    sp.wait_ge(act, 2 * tt + 3)  # both Signs(tt) done
                    sp.dma_start(out=o_d[tt], in_=osl(tt)).then_inc(do[tt % OS], 16)
            for t in range(T - LAG, T - 1):
                sp.wait_ge(act, 2 * t + 3)
                sp.dma_start(out=o_d[t], in_=osl(t)).then_inc(do[t % OS], 16)
            # last step: three stores pipelined with the split Signs
            t = T - 1
            h = d // 2
            sp.wait_ge(act, 2 * t + 2)  # Sign half a
            sp.dma_start(out=o_d[t][:, 0:h], in_=osl(t, 0, h)).then_inc(do[t % OS], 16)
            sp.wait_ge(act, 2 * t + 3)  # Sign half b
            sp.dma_start(out=o_d[t][:, h:d], in_=osl(t, h, d)).then_inc(do[t % OS], 16)
            sp.wait_ge(act, 2 * t + 4)  # Sign_s
            sp.dma_start(out=o_d[t][:, d:FD], in_=osl(t, d, FD)).then_inc(do[t % OS], 16)
            for i in range(OS):
                n_dmas = len([t for t in range(T) if t % OS == i])
                n_dmas += 2 if (T - 1) % OS == i else 0
                sp.wait_ge(do[i], 16 * n_dmas)

        @block.scalar
        def _(ac):
            ac.wait_ge(dw, 16)
            ac.activation(NT, WT, AF.Tanh, scale=-1.0)  # tanh odd: -tanh(w)
            ac.activation(TH, WT, AF.Tanh).then_inc(act, 1)
            ac.drain()
            for t in range(T):
                if t == T - 1:
                    # tail: sign each TT-U half as it lands, pool slice last
                    h = d // 2
                    ac.wait_ge(udve, t + 1)
                    ac.activation(osl(t, 0, h), usl(t, 0, h), AF.Sign, bias=NT).then_inc(
                        act, 1
                    )
                    ac.wait_ge(udve, t + 2)
                    ac.activation(osl(t, h, d), usl(t, h, d), AF.Sign, bias=NT).then_inc(
                        act, 1
                    )
                    ac.wait_ge(pc[1], 2 * t + 1)
                    ac.activation(
                        osl(t, d, FD), usl(t, d, FD), AF.Sign, bias=NT
                    ).then_inc(act, 1)
                    continue
                ac.wait_ge(udve, t + 1)  # U[:, :d](t) ready
                ac.activation(osl(t, 0, d), usl(t, 0, d), AF.Sign, bias=NT).then_inc(
                    act, 1
                )
                ac.wait_ge(pc[1], 2 * t + 1)  # U[:, d:](t) ready
                ac.activation(osl(t, d, FD), usl(t, d, FD), AF.Sign, bias=NT).then_inc(
                    act, 1
                )

        @block.vector
        def _(dv):
            dv.memset(Cc[:, 0:d], 0.0)
            dv.wait_ge(act, 1)  # thre ready
            for t in range(T):
                dv.wait_ge(dx[t % XS], 16 * (t // XS + 1))  # X(t) loaded
                if t >= US:
                    dv.wait_ge(act, 2 * (t - US) + 2)  # Sign_d(t-US) read U slot
                if t == T - 1:
                    # tail: split the add so each half's Sign can start sooner;
                    # z and the carry mult are dead on the last step
                    h = d // 2
                    dv.tensor_tensor(
                        usl(t, 0, h), Cc[:, 0:h], xsl(t, 0, h), AT.add
                    ).then_inc(udve, 1)
                    dv.tensor_tensor(
                        usl(t, h, d), Cc[:, h:d], xsl(t, h, d), AT.add
                    ).then_inc(udve, 1)
                    continue
                dv.tensor_tensor(usl(t, 0, d), Cc[:, 0:d], xsl(t, 0, d), AT.add).then_inc(
                    udve, 1
                )
                # merged z over [0, c1): d-slice + pool chunk0.  pc[i] >= 2t+1
                # means pool U_i(t) is done, which also implies C_i(t-1) has
                # read its Z chunk (pool program order), so one wait covers
                # both the RAW (U ready) and WAR (Z reusable) hazards.
                dv.wait_ge(pc[0], 2 * t + 1)
                dv.tensor_scalar(
                    Z[:, 0:c1], usl(t, 0, c1), TH, TAU, AT.is_le, AT.mult
                ).then_inc(zd[0], 1)
                dv.wait_ge(pc[1], 2 * t + 1)
                dv.tensor_scalar(
                    Z[:, c1:FD], usl(t, c1, FD), TH, TAU, AT.is_le, AT.mult
                ).then_inc(zd[1], 1)
                dv.tensor_tensor(Cc[:, 0:d], Z[:, 0:d], usl(t, 0, d), AT.mult)

        @block.gpsimd
        def _(pl):
            pl.memset(Cc[:, d:FD], 0.0)
            for t in range(T):
                pl.wait_ge(dx[t % XS], 16 * (t // XS + 1))  # X(t) loaded
                for i in range(2):
                    lo, hi = cuts[i], cuts[i + 1]
                    if t >= US:
                        pl.wait_ge(act, 2 * (t - US) + 3)  # Sign_s(t-US) read U
                        pl.wait_ge(zd[i], t - US + 1)  # z_i(t-US) read U slot
                    pl.tensor_tensor(
                        usl(t, lo, hi), Cc[:, lo:hi], xsl(t, lo, hi), AT.add
                    ).then_inc(pc[i], 1)
                if t == T - 1:
                    break  # the last carry is dead
                for i in range(2):
                    lo, hi = cuts[i], cuts[i + 1]
                    pl.wait_ge(zd[i], t + 1)  # z_i(t) ready
                    pl.tensor_tensor(
                        Cc[:, lo:hi], Z[:, lo:hi], usl(t, lo, hi), AT.mult
                    ).then_inc(pc[i], 1)

    return nc


def _get_nc():
    if "nc" not in _cache:
        _cache["nc"] = _build_nc()
    return _cache["nc"]


def _shard_x(x):
    """x [T,B,C,H,W] fp32 -> list of 8 contiguous [T,128,2048] fp16 arrays."""
    xf = x.reshape(T, B, C, HWF)
    shards = []
    for i in range(N_CORES):
        xc = xf[:, i * B_PER : (i + 1) * B_PER]  # [T,4,C,1024]
        xc = xc.reshape(T, 2, 2, C, HWF).transpose(0, 1, 3, 2, 4)  # t,bp,c,bf,f
        shards.append(np.ascontiguousarray(xc).reshape(T, P, FD).astype(np.float16))
    return shards


def _unshard_o(per_core):
    """list of 8 [T,128,2048] int8 sign values -> [T,B,C,H,W] fp32 spikes."""
    outs = []
    for oc in per_core:
        oc = (oc == 1).astype(np.float32)
        oc = oc.reshape(T, 2, C, 2, HWF).transpose(0, 1, 3, 2, 4)  # t,bp,bf,c,f
        outs.append(oc.reshape(T, B_PER, C, H, W))
    return np.concatenate(outs, axis=1)


def kernel(x, w):
    global last_results
    x = np.ascontiguousarray(np.asarray(x), dtype=np.float32)
    w = np.tile(np.asarray(w, dtype=np.float32).reshape(64, 1), (2, 1))  # [128,1]

    nc = _get_nc()
    shards = _shard_x(x)
    in_maps = [{"x": shards[i], "w": w} for i in range(N_CORES)]
    last_results = run_bass_kernel_spmd(nc, in_maps, core_ids=list(range(N_CORES)))
    return _unshard_o([last_results.results[i]["o"] for i in range(N_CORES)])


# revision 17
# speedup vs baseline: 1.8356x; 1.0191x over previous
"""LIF spike kernel (T-step leaky integrate-and-fire recurrence) on 8 TRN2 cores.

Reference semantics (per element, thre = tanh(w[c])):
    u_t = TAU * u_{t-1} * (1 - o_{t-1}) + x_t
    o_t = (u_t - thre > 0) ? 1.0 : 0.0

This version trades bit-exactness for speed (harness gate is rel_err < 2e-2):
x and the carried state are fp16, which measures rel_err ~1.1e-2 on the
reference inputs and unlocks the DVE 2x/4x perf modes plus half the input DMA
traffic vs fp32.

Per step, with carry C = TAU*u*(u<=thre), the fp32 STT pair of the baseline is
replaced by a 3-op fp16 form (fp16 tensor_tensor runs at 2x and tensor_scalar
at 4x, while scalar_tensor_tensor is stuck at 1x):
    TT : U = C + X_t                 (tensor_tensor add)
    TS : z = (U <= thre) * TAU       (tensor_scalar; z in {0, TAU})
    TT : C = z * U                   (TAU = 2^-2 so the scale is exact)
Work split: DVE runs TT-add/TS/TT-mult on cols [0,DCOL) plus the TS compares
for everything; the gpsimd/Pool engine runs the TT add AND TT mult for cols
[DCOL,2048) (walrus only accepts add/mult TTs on Pool) in 2 chunks, ordered
adds-first then mults so the pool->DVE z handoff pipelines.  ACT computes
spikes as two Sign ops (one per slice): O = Sign(U - thre) -> int8 {-1,0,1};
the host maps spike = (O == 1).  O stores are issued from the SP queue lagged
LAG steps behind the X loads so their semaphore waits never stall the load
prefetch.  Semaphores are merged (pool U/C share pc[i]; both Signs share act)
to minimize per-step wait count on the DVE sequencer.

Boundary-step specializations:
  t=0:  U(0) == X(0), so the identity adds and both carry memsets are skipped
        entirely -- z, the C mults, and the Signs read the X tile directly.
  t=15: the carry is dead; the TT-add is split into halves with per-half udve
        increments so ACT signs each half as it lands (pool slice last, since
        its U arrives late), and three per-slice stores pipeline with them.

Sharding: B=32 split across 8 cores (4 each).  Per-core SBUF layout:
partition p = bp*64 + c (bp = batch pair, c = channel), free f = bf*1024 + hw,
with b = bp*2 + bf.  The host pre-transposes x to fp16 so each timestep is one
contiguous [128, 2048] DMA; spikes return as int8 and are mapped on host.
"""

import numpy as np

import concourse.bass as bass
import concourse.mybir as mybir
from concourse.bass_utils import run_bass_kernel_spmd

TAU = 0.25
T, B, C, H, W = 16, 32, 64, 32, 32
N_CORES = 8
B_PER = B // N_CORES  # 4
HWF = H * W  # 1024
P = 128  # partitions: 2 batch-pairs x 64 channels
FD = (B_PER // 2) * HWF  # 2048 free-dim elements per partition per step

DCOL = 1608  # DVE-only slice; cols [DCOL, FD) are computed by Pool (+DVE TS)
XS = 4  # X buffer slots
US = 2  # U slots
OS = 16  # O slots (one per step: no store-slot recycling waits)
LAG = 3  # store O(t-LAG) after loading X(t) on the SP queue

_cache = {}
last_results = None  # BassKernelResults of the most recent run (for test harness)


def _build_nc():
    nc = bass.Bass("TRN2", target_bir_lowering=False, debug=False, num_devices=N_CORES)
    f32 = mybir.dt.float32
    f16 = mybir.dt.float16
    i8 = mybir.dt.int8
    x_d = nc.dram_tensor("x", [T, P, FD], f16, kind="ExternalInput").ap()
    w_d = nc.dram_tensor("w", [P, 1], f32, kind="ExternalInput").ap()
    o_d = nc.dram_tensor("o", [T, P, FD], i8, kind="ExternalOutput").ap()

    AT = mybir.AluOpType
    AF = mybir.ActivationFunctionType

    d = DCOL
    s = FD - d
    c1 = d + (s * 45) // 100
    cuts = [d, c1, FD]
    h = d // 2

    X = nc.alloc_sbuf_tensor("Xb", [P, XS * FD], f16).ap()
    U = nc.alloc_sbuf_tensor("Ub", [P, US * FD], f16).ap()
    Cc = nc.alloc_sbuf_tensor("Cb", [P, FD], f16).ap()
    Z = nc.alloc_sbuf_tensor("Zb", [P, FD], f16).ap()
    O = nc.alloc_sbuf_tensor("Ob", [P, OS * FD], i8).ap()
    WT = nc.alloc_sbuf_tensor("WTb", [P, 1], f32).ap()
    NT = nc.alloc_sbuf_tensor("NTb", [P, 1], f32).ap()  # -tanh(w)
    TH = nc.alloc_sbuf_tensor("THb", [P, 1], f32).ap()  # +tanh(w)

    def xsl(t, lo, hi):
        return X[:, (t % XS) * FD + lo : (t % XS) * FD + hi]

    def usl(t, lo, hi):
        return U[:, (t % US) * FD + lo : (t % US) * FD + hi]

    def osl(t, lo=0, hi=FD):
        return O[:, (t % OS) * FD + lo : (t % OS) * FD + hi]

    import contextlib

    with contextlib.ExitStack() as st:
        block = st.enter_context(nc.Block())
        # udve: TT_U_d(t) -> t for t in 1..14; t=15 halves -> 15, 16
        udve = st.enter_context(nc.semaphore("udve"))
        # act: +1 tanh; +2 per step (Sign_d then Sign_s) -> 2t+3 after step t
        act = st.enter_context(nc.semaphore("act"))
        dw = st.enter_context(nc.semaphore("dw"))
        # pc[i]: pool chunk i; C_i(0) -> 1; then U_i(t) -> 2t, C_i(t) -> 2t+1
        pc = [st.enter_context(nc.semaphore(f"pc{i}")) for i in range(2)]
        zd = [st.enter_context(nc.semaphore(f"zd{i}")) for i in range(2)]
        dx = [st.enter_context(nc.semaphore(f"dx{i}")) for i in range(XS)]
        do = [st.enter_context(nc.semaphore(f"do{i}")) for i in range(OS)]

        @block.sync
        def _(sp):
            for t in range(T):
                if t == XS:
                    # slot 0's step-0 readers worked on the X tile directly
                    sp.wait_ge(udve, 1)  # TT_U_d(1) done => DVE step-0 reads done
                    sp.wait_ge(pc[1], 1)  # pool C2(0) read xsl(0)
                    sp.wait_ge(act, 3)  # Signs(0) read xsl(0)
                elif t > XS:
                    sp.wait_ge(udve, t - XS)
                    sp.wait_ge(pc[1], 2 * (t - XS))
                sp.dma_start(out=xsl(t, 0, FD), in_=x_d[t]).then_inc(dx[t % XS], 16)
                if t == 0:
                    sp.dma_start(out=WT, in_=w_d).then_inc(dw, 16)
                if t >= LAG:
                    tt = t - LAG
                    sp.wait_ge(act, 2 * tt + 3)  # both Signs(tt) done
                    sp.dma_start(out=o_d[tt], in_=osl(tt)).then_inc(do[tt % OS], 16)
            for t in range(T - LAG, T - 1):
                sp.wait_ge(act, 2 * t + 3)
                sp.dma_start(out=o_d[t], in_=osl(t)).then_inc(do[t % OS], 16)
            # last step: three stores pipelined with the split Signs
            t = T - 1
            sp.wait_ge(act, 2 * t + 2)  # Sign half a
            sp.dma_start(out=o_d[t][:, 0:h], in_=osl(t, 0, h)).then_inc(do[t % OS], 16)
            sp.wait_ge(act, 2 * t + 3)  # Sign half b
            sp.dma_start(out=o_d[t][:, h:d], in_=osl(t, h, d)).then_inc(do[t % OS], 16)
            sp.wait_ge(act, 2 * t + 4)  # Sign_s
            sp.dma_start(out=o_d[t][:, d:FD], in_=osl(t, d, FD)).then_inc(do[t % OS], 16)
            for i in range(OS):
                n_dmas = len([t for t in range(T) if t % OS == i])
                n_dmas += 2 if (T - 1) % OS == i else 0
                sp.wait_ge(do[i], 16 * n_dmas)

        @block.scalar
        def _(ac):
            ac.wait_ge(dw, 16)
            ac.activation(NT, WT, AF.Tanh, scale=-1.0)  # tanh odd: -tanh(w)
            ac.activation(TH, WT, AF.Tanh).then_inc(act, 1)
            ac.drain()
            for t in range(T):
                if t == 0:
                    ac.wait_ge(dx[0], 16)
                    ac.activation(osl(0, 0, d), xsl(0, 0, d), AF.Sign, bias=NT).then_inc(
                        act, 1
                    )
                    ac.activation(
                        osl(0, d, FD), xsl(0, d, FD), AF.Sign, bias=NT
                    ).then_inc(act, 1)
                    continue
                if t == T - 1:
                    ac.wait_ge(udve, t)
                    ac.activation(osl(t, 0, h), usl(t, 0, h), AF.Sign, bias=NT).then_inc(
                        act, 1
                    )
                    ac.wait_ge(udve, t + 1)
                    ac.activation(osl(t, h, d), usl(t, h, d), AF.Sign, bias=NT).then_inc(
                        act, 1
                    )
                    ac.wait_ge(pc[1], 2 * t)
                    ac.activation(
                        osl(t, d, FD), usl(t, d, FD), AF.Sign, bias=NT
                    ).then_inc(act, 1)
                    continue
                ac.wait_ge(udve, t)  # U[:, :d](t) ready
                ac.activation(osl(t, 0, d), usl(t, 0, d), AF.Sign, bias=NT).then_inc(
                    act, 1
                )
                ac.wait_ge(pc[1], 2 * t)  # U[:, d:](t) ready
                ac.activation(osl(t, d, FD), usl(t, d, FD), AF.Sign, bias=NT).then_inc(
                    act, 1
                )

        @block.vector
        def _(dv):
            dv.wait_ge(act, 1)  # thre ready
            for t in range(T):
                dv.wait_ge(dx[t % XS], 16 * (t // XS + 1))  # X(t) loaded
                if t >= US:
                    dv.wait_ge(act, 2 * (t - US) + 2)  # Sign_d(t-US) read U slot
                if t == 0:
                    # U(0) == X(0): skip the identity add, read the X tile
                    dv.tensor_scalar(
                        Z[:, 0:c1], xsl(0, 0, c1), TH, TAU, AT.is_le, AT.mult
                    ).then_inc(zd[0], 1)
                    dv.tensor_scalar(
                        Z[:, c1:FD], xsl(0, c1, FD), TH, TAU, AT.is_le, AT.mult
                    ).then_inc(zd[1], 1)
                    dv.tensor_tensor(Cc[:, 0:d], Z[:, 0:d], xsl(0, 0, d), AT.mult)
                    continue
                if t == T - 1:
                    # tail: split the add so each half's Sign starts sooner;
                    # z and the carry mult are dead on the last step
                    dv.tensor_tensor(
                        usl(t, 0, h), Cc[:, 0:h], xsl(t, 0, h), AT.add
                    ).then_inc(udve, 1)
                    dv.tensor_tensor(
                        usl(t, h, d), Cc[:, h:d], xsl(t, h, d), AT.add
                    ).then_inc(udve, 1)
                    continue
                dv.tensor_tensor(usl(t, 0, d), Cc[:, 0:d], xsl(t, 0, d), AT.add).then_inc(
                    udve, 1
                )
                # merged z over [0, c1): d-slice + pool chunk0.  pc[i] >= 2t
                # means pool U_i(t) is done, which (pool program order) also
                # implies C_i(t-1) has read its Z chunk: one wait covers both
                # the RAW (U ready) and WAR (Z reusable) hazards.
                dv.wait_ge(pc[0], 2 * t)
                dv.tensor_scalar(
                    Z[:, 0:c1], usl(t, 0, c1), TH, TAU, AT.is_le, AT.mult
                ).then_inc(zd[0], 1)
                dv.wait_ge(pc[1], 2 * t)
                dv.tensor_scalar(
                    Z[:, c1:FD], usl(t, c1, FD), TH, TAU, AT.is_le, AT.mult
                ).then_inc(zd[1], 1)
                dv.tensor_tensor(Cc[:, 0:d], Z[:, 0:d], usl(t, 0, d), AT.mult)

        @block.gpsimd
        def _(pl):
            for t in range(T):
                pl.wait_ge(dx[t % XS], 16 * (t // XS + 1))  # X(t) loaded
                if t >= 1:
                    for i in range(2):
                        lo, hi = cuts[i], cuts[i + 1]
                        if t >= US:
                            pl.wait_ge(act, 2 * (t - US) + 3)  # Sign_s(t-US)
                            pl.wait_ge(zd[i], t - 1)  # z_i(t-US) read U slot
                        pl.tensor_tensor(
                            usl(t, lo, hi), Cc[:, lo:hi], xsl(t, lo, hi), AT.add
                        ).then_inc(pc[i], 1)
                if t == T - 1:
                    break  # the last carry is dead
                for i in range(2):
                    lo, hi = cuts[i], cuts[i + 1]
                    pl.wait_ge(zd[i], t + 1)  # z_i(t) ready
                    srcu = xsl(0, lo, hi) if t == 0 else usl(t, lo, hi)
                    pl.tensor_tensor(
                        Cc[:, lo:hi], Z[:, lo:hi], srcu, AT.mult
                    ).then_inc(pc[i], 1)

    return nc


def _get_nc():
    if "nc" not in _cache:
        _cache["nc"] = _build_nc()
    return _cache["nc"]


def _shard_x(x):
    """x [T,B,C,H,W] fp32 -> list of 8 contiguous [T,128,2048] fp16 arrays."""
    xf = x.reshape(T, B, C, HWF)
    shards = []
    for i in range(N_CORES):
        xc = xf[:, i * B_PER : (i + 1) * B_PER]  # [T,4,C,1024]
        xc = xc.reshape(T, 2, 2, C, HWF).transpose(0, 1, 3, 2, 4)  # t,bp,c,bf,f
        shards.append(np.ascontiguousarray(xc).reshape(T, P, FD).astype(np.float16))
    return shards


def _unshard_o(per_core):
    """list of 8 [T,128,2048] int8 sign values -> [T,B,C,H,W] fp32 spikes."""
    outs = []
    for oc in per_core:
        oc = (oc == 1).astype(np.float32)
        oc = oc.reshape(T, 2, C, 2, HWF).transpose(0, 1, 3, 2, 4)  # t,bp,bf,c,f
        outs.append(oc.reshape(T, B_PER, C, H, W))
    return np.concatenate(outs, axis=1)


def kernel(x, w):
    global last_results
    x = np.ascontiguousarray(np.asarray(x), dtype=np.float32)
    w = np.tile(np.asarray(w, dtype=np.float32).reshape(64, 1), (2, 1))  # [128,1]

    nc = _get_nc()
    shards = _shard_x(x)
    in_maps = [{"x": shards[i], "w": w} for i in range(N_CORES)]
    last_results = run_bass_kernel_spmd(nc, in_maps, core_ids=list(range(N_CORES)))
    return _unshard_o([last_results.results[i]["o"] for i in range(N_CORES)])
